# revision 26
# baseline (speedup 1.0000x reference)
"""Trainium2 Bass kernel for GQA attention (B=4, S=1024, D=4096, HQ=32, HKV=8).

Sharding: 8 cores = 4 batches x 2 head-groups. Each core computes one batch
with 16 q-heads / 4 kv-heads (Wq/Wk/Wv column-sharded, Wo row-sharded). The
two head-group partial outputs per batch are summed on the host (this is the
Wo-row-shard reduction, done host-side instead of an on-device all-reduce),
then transposed (device emits out^T [Dout, S]) and bias bo added.

Device dataflow per core (SPMD, identical graph):
  Phase 1 (QKV): q[s,dq] = xT_chunk.T @ Wq_chunk (bf16, psum accumulate over
    D), evict + bias via DVE; RoPE on DVE in [s, d] layout using host-permuted
    "evens-first" head columns; PE-transpose q,k -> qT,kT [d, s] (bf16).
  Phase 2 (attention, per head): scores_i = qT_i.T @ kT (causal), diag-block
    mask added in psum, exp+rowsum fused on ScalarE (unsafe softmax - no max
    subtraction; scores are O(30) so fp32 exp is safe), normalize on DVE,
    PE-transpose attn -> attnT, av: oT += v_j.T @ attnT_j.
  Phase 3: outT = Wo_chunk.T @ oT (accumulate over dq), DMA out^T.
"""

import math
import os

import numpy as np
import ml_dtypes

import concourse.bass as bass
import concourse.mybir as mybir
import concourse.tile as tile
from concourse import bacc
from concourse.bass_utils import run_bass_kernel_spmd
from concourse.masks import make_causal_mask, make_identity

B, S, D = 4, 1024, 4096
HQ, HKV, HD = 32, 8, 128
NH = 16          # q heads per core
NKV = 4          # kv heads per core
DQ = NH * HD     # 2048
DK = NKV * HD    # 512
NDC = D // 128   # 32 D-chunks
NSC = S // 128   # 8 s-chunks
QK_SCALE = 1.0 / math.sqrt(HD)
MASK_VAL = -1e30

F32 = mybir.dt.float32
BF16 = mybir.dt.bfloat16

_GRAPH_CACHE = {}
LAST_PROFILE = None


def _bcast_like(small_ap, big_ap):
    """Broadcast small_ap (size-1 dims) against big_ap's shape."""
    a, b = bass.broadcast_tensor_aps(big_ap, small_ap)
    return b


def _build_graph():
    nc = bacc.Bacc(debug=False)

    xt_ext = nc.dram_tensor("xt", [NDC, 128, S], BF16, kind="ExternalInput")
    # packed q|k|v weight columns (host-permuted evens-first for q/k)
    wqkv_ext = nc.dram_tensor("wqkv", [NDC, 128, DQ + DK + DK], BF16,
                              kind="ExternalInput")
    wo_ext = nc.dram_tensor("wo", [NH, 128, D], BF16, kind="ExternalInput")
    fcc_ext = nc.dram_tensor("fcc", [128, NSC, 64], F32, kind="ExternalInput")
    fcd_ext = nc.dram_tensor("fcd", [128, NSC, 64], F32, kind="ExternalInput")
    # bias column tile: col j = bias for output-column-block j (q0..q15,k0..3,v0..3)
    bqkv_ext = nc.dram_tensor("bqkv", [128, 24], F32, kind="ExternalInput")
    out_ext = nc.dram_tensor("out", [D, S], F32, kind="ExternalOutput")

    with tile.TileContext(nc) as tc:
        with (
            tc.tile_pool(name="const", bufs=1) as cpool,
            tc.tile_pool(name="persist", bufs=1) as ppool,
            tc.tile_pool(name="stat", bufs=2) as spool,
        ):
            # maskT for scoresT [s2, s1]: keep s1 >= s2 (0), else -1e30
            maskT = cpool.tile([128, 128], F32)
            nc.gpsimd.memset(maskT[:], 0.0)
            nc.gpsimd.affine_select(
                out=maskT[:], in_=maskT[:], compare_op=mybir.AluOpType.is_ge,
                fill=MASK_VAL, base=0, pattern=[[1, 128]], channel_multiplier=-1)
            ident_b = cpool.tile([128, 128], BF16)
            make_identity(nc, ident_b)
            ones_col = cpool.tile([128, 1], BF16)   # den matmul lhsT
            nc.gpsimd.memset(ones_col[:], 1.0)
            ones_row = cpool.tile([1, 128], F32)    # bcast matmul lhsT (K=1)
            nc.gpsimd.memset(ones_row[:], 1.0)
            fcc_sb = cpool.tile([128, NSC, 64], F32)
            nc.sync.dma_start(out=fcc_sb[:], in_=fcc_ext[:])
            fcd_sb = cpool.tile([128, NSC, 64], F32)
            nc.sync.dma_start(out=fcd_sb[:], in_=fcd_ext[:])
            bias_sb = cpool.tile([128, 24], F32)
            nc.sync.dma_start(out=bias_sb[:], in_=bqkv_ext[:])

            # persistent activations
            qT_all = ppool.tile([128, NH * S], BF16)    # [d, h*S + s]
            kT_all = ppool.tile([128, NKV * S], BF16)   # [d, g*S + s]
            v_all = ppool.tile([128, NSC * DK], BF16)   # [s2 in chunk, sc*DK + d]

            # ---------------- Phase 1: QKV + rope + transpose ----------------
            # W-stationary: for each output-column block (24 blocks of 128:
            # 16 q-heads, 4 k-heads, 4 v-heads) accumulate psum [128, S] over
            # the 32 D-chunks with x (bf16, SBUF-resident) as moving operand.
            # Output arrives transposed ([d, s]); q/k are PE-transposed to
            # [s, d] per s-chunk for full-lane rope on DVE, then transposed
            # back into qT/kT. v is transposed once into [s2, d] layout.
            with (
                tc.tile_pool(name="p1sb", bufs=1) as p1,
                tc.tile_pool(name="p1ps", bufs=1, space="PSUM") as ps1,
            ):
                xts = []
                for dc in range(NDC):
                    xt_t = p1.tile([128, S], BF16, tag=f"xt{dc}", name=f"xt{dc}")
                    nc.sync.dma_start(out=xt_t[:], in_=xt_ext[dc])
                    xts.append(xt_t)

                # qk_sb[sc]: roped q|k in [s, d] layout, assembled then roped
                qk_sb = []
                for sc in range(NSC):
                    t = p1.tile([128, DQ + DK], BF16, tag=f"qk{sc}", name=f"qk{sc}")
                    qk_sb.append(t)

                def emit_rope(col_off, nh, sfx):
                    # rope per s-chunk on cols [col_off, col_off + nh*128)
                    for sc in range(NSC):
                        t3 = qk_sb[sc][:, col_off:col_off + nh * 128].rearrange(
                            "p (h c) -> p h c", c=128)
                        E = t3[:, :, 0:64]
                        O = t3[:, :, 64:128]
                        Cb = _bcast_like(fcc_sb[:, sc:sc + 1, :], E)
                        Db = _bcast_like(fcd_sb[:, sc:sc + 1, :], E)
                        tmps = []
                        for idx, (a, bb) in enumerate(
                                ((E, Cb), (O, Db), (E, Db), (O, Cb))):
                            t = p1.tile([128, nh, 64], BF16, tag=f"rt{idx}",
                                        bufs=2, name=f"rt{idx}_{sc}{sfx}")
                            nc.vector.tensor_tensor(out=t[:], in0=a, in1=bb,
                                                    op=mybir.AluOpType.mult)
                            tmps.append(t)
                        nc.vector.tensor_tensor(out=E, in0=tmps[0][:], in1=tmps[1][:],
                                                op=mybir.AluOpType.subtract)
                        nc.vector.tensor_tensor(out=O, in0=tmps[2][:], in1=tmps[3][:],
                                                op=mybir.AluOpType.add)

                def emit_back(col_off, n, dstT, dst_off, sfx):
                    # transpose roped [s, d] cols back into dstT [d, s] layout
                    for sc in range(NSC):
                        for hh in range(n):
                            tp = ps1.tile([128, 128], BF16, tag="tp", bufs=2,
                                          name=f"tb{sc}_{hh}{sfx}")
                            nc.tensor.transpose(
                                tp[:],
                                qk_sb[sc][:, col_off + hh * 128:
                                          col_off + (hh + 1) * 128],
                                ident_b)
                            nc.scalar.copy(
                                dstT[:, (dst_off + hh) * S + sc * 128:
                                     (dst_off + hh) * S + (sc + 1) * 128],
                                tp[:])

                # 24 column-blocks in order [k0..3, v0..3, q0..15], groups of
                # 3; the group's weight columns [128, 384] stream per D-chunk.
                # k/v first so rope-k and attention deps resolve early; rope
                # is emitted as soon as its source blocks are complete.
                for grp in range(8):
                    accs = [
                        ps1.tile([128, S], F32, tag="acc", bufs=3, name=f"acc{grp}_{b}")
                        for b in range(3)
                    ]
                    for dc in range(NDC):
                        w_t = p1.tile([128, 384], BF16, tag="w", bufs=4,
                                      name=f"w{grp}_{dc}")
                        nc.sync.dma_start(
                            out=w_t[:],
                            in_=wqkv_ext[dc, :, grp * 384:(grp + 1) * 384])
                        for b in range(3):
                            lhs = w_t[:, b * 128:(b + 1) * 128]
                            for c0 in range(0, S, 512):
                                nc.tensor.matmul(
                                    accs[b][:, c0:c0 + 512], lhs,
                                    xts[dc][:, c0:c0 + 512],
                                    start=(dc == 0), stop=(dc == NDC - 1))
                    for b in range(3):
                        blk = grp * 3 + b
                        # evict + bias (per-partition col) -> bf16 [d, s]
                        tf = p1.tile([128, S], BF16, tag="tf", bufs=3,
                                     name=f"tf{blk}")
                        nc.vector.tensor_scalar(
                            out=tf[:], in0=accs[b][:],
                            scalar1=bias_sb[:, blk:blk + 1],
                            scalar2=None, op0=mybir.AluOpType.add)
                        if blk < 4:          # k block
                            col = DQ + blk * 128
                        elif blk < 8:        # v block
                            col = None
                            g = blk - 4
                        else:                # q block
                            col = (blk - 8) * 128
                        for sc in range(NSC):
                            tp = ps1.tile([128, 128], BF16, tag="tp", bufs=2,
                                          name=f"tp{blk}_{sc}")
                            nc.tensor.transpose(
                                tp[:], tf[:, sc * 128:(sc + 1) * 128], ident_b)
                            if col is not None:
                                nc.scalar.copy(qk_sb[sc][:, col:col + 128], tp[:])
                            else:
                                nc.scalar.copy(
                                    v_all[:, sc * DK + g * 128:
                                          sc * DK + (g + 1) * 128],
                                    tp[:])
                    if grp == 1:   # k0..k3 assembled
                        emit_rope(DQ, NKV, "k")
                        emit_back(DQ, NKV, kT_all, 0, "k")
                    elif grp == 5:  # q0..q7 assembled
                        emit_rope(0, 8, "qlo")
                        emit_back(0, 8, qT_all, 0, "qlo")
                    elif grp == 7:  # q8..q15 assembled
                        emit_rope(8 * 128, 8, "qhi")
                        emit_back(8 * 128, 8, qT_all, 8, "qhi")

            # ---------------- Phase 2: attention ----------------
            # oT_all lives through phases 2+3; released manually at the end.
            p23 = tc.alloc_tile_pool(name="p23sb", bufs=1)
            oT_all = p23.tile([128, NH * S], BF16, name="oT_all")  # [d, h*S+s]
            with (
                tc.tile_pool(name="p2sb", bufs=1) as p2,
                tc.tile_pool(name="p2ps", bufs=1, space="PSUM") as ps2,
            ):
                for h in range(NH):
                    g = h // 4
                    # scoresT_j = kT_j.T @ qT (causal: s1 >= j*128), exp -> aT,
                    # then av + den accumulate immediately
                    otp = ps2.tile([128, 1024], F32, tag="ot", bufs=2, name=f"otp{h}")
                    den = ps2.tile([1, 1024], F32, tag="ot", bufs=2, name=f"den{h}")
                    for j in range(NSC):
                        if j < 4:
                            chunks = [(j * 128, 512), (512, 1024)]
                        else:
                            chunks = [(j * 128, 1024)]
                        scp = ps2.tile([128, 1024], F32, tag="sc", bufs=2,
                                       name=f"scp{h}_{j}")
                        lhs = kT_all[:, g * S + j * 128: g * S + (j + 1) * 128]
                        for (c0, c1) in chunks:
                            nc.tensor.matmul(
                                scp[:, c0:c1], lhs,
                                qT_all[:, h * S + c0: h * S + c1],
                                start=True, stop=True)
                        # causal mask on diagonal block (in-place psum add)
                        nc.vector.tensor_tensor(
                            out=scp[:, j * 128:(j + 1) * 128],
                            in0=scp[:, j * 128:(j + 1) * 128],
                            in1=maskT[:], op=mybir.AluOpType.add)
                        aT = p2.tile([128, 1024], BF16, tag="aT", bufs=3,
                                     name=f"aT{h}_{j}")
                        nc.scalar.activation(
                            aT[:, j * 128:1024], scp[:, j * 128:1024],
                            mybir.ActivationFunctionType.Exp, scale=QK_SCALE)
                        vs = v_all[:, j * DK + g * 128: j * DK + (g + 1) * 128]
                        for (c0, c1) in chunks:
                            nc.tensor.matmul(
                                otp[:, c0:c1], vs, aT[:, c0:c1],
                                start=(j == 0), stop=(j == NSC - 1),
                                skip_group_check=True)
                            nc.tensor.matmul(
                                den[:, c0:c1], ones_col[:], aT[:, c0:c1],
                                start=(j == 0), stop=(j == NSC - 1),
                                skip_group_check=True)

                    # normalize: recip row, PE-broadcast to 128 partitions,
                    # multiply during psum->sbuf eviction
                    rrow = spool.tile([1, 1024], F32, tag="rrow", name=f"rr{h}")
                    nc.vector.reciprocal(rrow[:], den[:])
                    rb = ps2.tile([128, 1024], F32, tag="sc", bufs=2, name=f"rb{h}")
                    for c0 in range(0, S, 512):
                        nc.tensor.matmul(rb[:, c0:c0 + 512], ones_row[:],
                                         rrow[:, c0:c0 + 512],
                                         start=True, stop=True)
                    rbs = p2.tile([128, 1024], F32, tag="rbs", bufs=2, name=f"rbs{h}")
                    nc.scalar.copy(rbs[:], rb[:])
                    nc.vector.tensor_tensor(
                        out=oT_all[:, h * S:(h + 1) * S], in0=otp[:], in1=rbs[:],
                        op=mybir.AluOpType.mult)

            # ---------------- Phase 3: Wo ----------------
            # Groups of 4 Dout-chunks x both s-halves (8 psum banks); each
            # stationary Wo block serves both halves, Wo is streamed once.
            with (
                tc.tile_pool(name="p3sb", bufs=1) as p3,
                tc.tile_pool(name="p3ps", bufs=1, space="PSUM") as ps3,
            ):
                for mg in range(8):   # groups of 4 Dout-chunks
                    wps = [
                        ps3.tile([128, 512], F32, tag="wps", bufs=8,
                                 name=f"wps{mg}_{i}")
                        for i in range(8)  # [m0s0, m0s1, m1s0, ...]
                    ]
                    for c in range(NH):
                        wo_t = p3.tile([128, 512], BF16, tag="wo", bufs=4,
                                       name=f"wo{mg}_{c}")
                        nc.sync.dma_start(
                            out=wo_t[:],
                            in_=wo_ext[c, :, mg * 512:(mg + 1) * 512])
                        for m in range(4):
                            lhs = wo_t[:, m * 128:(m + 1) * 128]
                            for sh in range(2):
                                rhs = oT_all[:, c * S + sh * 512:
                                             c * S + sh * 512 + 512]
                                nc.tensor.matmul(
                                    wps[m * 2 + sh][:], lhs, rhs,
                                    start=(c == 0), stop=(c == NH - 1))
                    for m in range(4):
                        for sh in range(2):
                            ot_sb = p3.tile([128, 512], F32, tag="ot_sb", bufs=4,
                                            name=f"osb{mg}_{m}_{sh}")
                            nc.vector.tensor_copy(ot_sb[:], wps[m * 2 + sh][:])
                            mm = mg * 4 + m
                            nc.sync.dma_start(
                                out=out_ext[mm * 128:(mm + 1) * 128,
                                            sh * 512:(sh + 1) * 512],
                                in_=ot_sb[:])
            p23.release()

    nc.compile()
    return nc


def _evens_first_perm(nheads):
    idx = []
    for h in range(nheads):
        base = h * HD
        idx.extend(range(base, base + HD, 2))
        idx.extend(range(base + 1, base + HD, 2))
    return np.array(idx, dtype=np.int64)


def kernel(x, freqs_cis, Wq, bq, Wk, bk, Wv, bv, Wo, bo, startpos):
    global LAST_PROFILE
    x = np.asarray(x, dtype=np.float32)
    freqs_cis = np.asarray(freqs_cis, dtype=np.float32)
    Wq = np.asarray(Wq, dtype=np.float32)
    Wk = np.asarray(Wk, dtype=np.float32)
    Wv = np.asarray(Wv, dtype=np.float32)
    Wo = np.asarray(Wo, dtype=np.float32)
    bq = np.asarray(bq, dtype=np.float32)
    bk = np.asarray(bk, dtype=np.float32)
    bv = np.asarray(bv, dtype=np.float32)
    bo = np.asarray(bo, dtype=np.float32)
    assert int(startpos) == 0

    bf = lambda a: np.ascontiguousarray(a.astype(ml_dtypes.bfloat16))
    f32c = lambda a: np.ascontiguousarray(a.astype(np.float32))

    fcc = f32c(freqs_cis[:, :, 0].reshape(NSC, 128, 64).transpose(1, 0, 2))
    fcd = f32c(freqs_cis[:, :, 1].reshape(NSC, 128, 64).transpose(1, 0, 2))

    in_maps = []
    for core in range(8):
        b, g = core // 2, core % 2
        qsel = g * DQ + _evens_first_perm(NH)
        ksel = g * DK + _evens_first_perm(NKV)
        vsel = np.arange(g * DK, (g + 1) * DK)
        if core < 2:  # weight shards depend only on g; reuse for later cores
            wqkv_h = bf(np.concatenate(
                [Wk[:, ksel], Wv[:, vsel], Wq[:, qsel]], 1
            ).reshape(NDC, 128, DQ + DK + DK))
            wo_h = bf(Wo[g * DQ:(g + 1) * DQ, :].reshape(NH, 128, D))
            bqkv = np.concatenate([bk[ksel], bv[vsel], bq[qsel]])
            bqkv = f32c(bqkv.reshape(24, 128).T)  # [128, 24]: col j = block j bias
        else:
            prev = in_maps[core - 2]
            wqkv_h, wo_h, bqkv = prev["wqkv"], prev["wo"], prev["bqkv"]
        xt_h = bf(x[b].T.reshape(NDC, 128, S))
        in_maps.append({
            "xt": xt_h, "wqkv": wqkv_h, "wo": wo_h,
            "fcc": fcc, "fcd": fcd, "bqkv": bqkv,
        })

    if "nc" not in _GRAPH_CACHE:
        _GRAPH_CACHE["nc"] = _build_graph()
    nc = _GRAPH_CACHE["nc"]

    res = run_bass_kernel_spmd(
        nc, in_maps, core_ids=list(range(8)),
        trace=bool(os.environ.get("BASS_TRACE")))
    LAST_PROFILE = res

    out = np.empty((B, S, D), dtype=np.float32)
    for b in range(B):
        t = res.results[2 * b]["out"] + res.results[2 * b + 1]["out"]
        out[b] = t.T + bo[None, :]
    return out


# revision 28
# speedup vs baseline: 1.0430x; 1.0430x over previous
"""Trainium2 Bass kernel for GQA attention (B=4, S=1024, D=4096, HQ=32, HKV=8).

Sharding: 8 cores = 4 batches x 2 head-groups. Each core computes one batch
with 16 q-heads / 4 kv-heads (Wq/Wk/Wv column-sharded, Wo row-sharded). The
two head-group partial outputs per batch are summed on the host (this is the
Wo-row-shard reduction, done host-side instead of an on-device all-reduce),
then transposed (device emits out^T [Dout, S]) and bias bo added.

Device dataflow per core (SPMD, identical graph):
  Phase 1 (QKV): q[s,dq] = xT_chunk.T @ Wq_chunk (bf16, psum accumulate over
    D), evict + bias via DVE; RoPE on DVE in [s, d] layout using host-permuted
    "evens-first" head columns; PE-transpose q,k -> qT,kT [d, s] (bf16).
  Phase 2 (attention, per head): scores_i = qT_i.T @ kT (causal), diag-block
    mask added in psum, exp+rowsum fused on ScalarE (unsafe softmax - no max
    subtraction; scores are O(30) so fp32 exp is safe), normalize on DVE,
    PE-transpose attn -> attnT, av: oT += v_j.T @ attnT_j.
  Phase 3: outT = Wo_chunk.T @ oT (accumulate over dq), DMA out^T.
"""

import math
import os

import numpy as np
import ml_dtypes

import concourse.bass as bass
import concourse.mybir as mybir
import concourse.tile as tile
from concourse import bacc
from concourse.bass_utils import run_bass_kernel_spmd
from concourse.masks import make_causal_mask, make_identity

B, S, D = 4, 1024, 4096
HQ, HKV, HD = 32, 8, 128
NH = 16          # q heads per core
NKV = 4          # kv heads per core
DQ = NH * HD     # 2048
DK = NKV * HD    # 512
NDC = D // 128   # 32 D-chunks
NSC = S // 128   # 8 s-chunks
QK_SCALE = 1.0 / math.sqrt(HD)
MASK_VAL = -1e30

F32 = mybir.dt.float32
BF16 = mybir.dt.bfloat16

_GRAPH_CACHE = {}
LAST_PROFILE = None


def _bcast_like(small_ap, big_ap):
    """Broadcast small_ap (size-1 dims) against big_ap's shape."""
    a, b = bass.broadcast_tensor_aps(big_ap, small_ap)
    return b


def _build_graph():
    nc = bacc.Bacc(debug=False)

    xt_ext = nc.dram_tensor("xt", [NDC, 128, S], BF16, kind="ExternalInput")
    # packed q|k|v weight columns (host-permuted evens-first for q/k)
    wqkv_ext = nc.dram_tensor("wqkv", [NDC, 128, DQ + DK + DK], BF16,
                              kind="ExternalInput")
    wo_ext = nc.dram_tensor("wo", [NH, 128, D], BF16, kind="ExternalInput")
    fcc_ext = nc.dram_tensor("fcc", [128, NSC, 64], F32, kind="ExternalInput")
    fcd_ext = nc.dram_tensor("fcd", [128, NSC, 64], F32, kind="ExternalInput")
    # bias column tile: col j = bias for output-column-block j (q0..q15,k0..3,v0..3)
    bqkv_ext = nc.dram_tensor("bqkv", [128, 24], F32, kind="ExternalInput")
    out_ext = nc.dram_tensor("out", [D, S], F32, kind="ExternalOutput")

    with tile.TileContext(nc) as tc:
        with (
            tc.tile_pool(name="const", bufs=1) as cpool,
            tc.tile_pool(name="persist", bufs=1) as ppool,
            tc.tile_pool(name="stat", bufs=2) as spool,
        ):
            # maskT for scoresT [s2, s1]: keep s1 >= s2 (0), else -1e30
            maskT = cpool.tile([128, 128], F32)
            nc.gpsimd.memset(maskT[:], 0.0)
            nc.gpsimd.affine_select(
                out=maskT[:], in_=maskT[:], compare_op=mybir.AluOpType.is_ge,
                fill=MASK_VAL, base=0, pattern=[[1, 128]], channel_multiplier=-1)
            ident_b = cpool.tile([128, 128], BF16)
            make_identity(nc, ident_b)
            ones_col = cpool.tile([128, 1], BF16)   # den matmul lhsT
            nc.gpsimd.memset(ones_col[:], 1.0)
            ones_row = cpool.tile([1, 128], BF16)   # bcast matmul lhsT (K=1)
            nc.gpsimd.memset(ones_row[:], 1.0)
            fcc_sb = cpool.tile([128, NSC, 64], F32)
            nc.sync.dma_start(out=fcc_sb[:], in_=fcc_ext[:])
            fcd_sb = cpool.tile([128, NSC, 64], F32)
            nc.sync.dma_start(out=fcd_sb[:], in_=fcd_ext[:])
            bias_sb = cpool.tile([128, 24], F32)
            nc.sync.dma_start(out=bias_sb[:], in_=bqkv_ext[:])

            # persistent activations
            qT_all = ppool.tile([128, NH * S], BF16)    # [d, h*S + s]
            kT_all = ppool.tile([128, NKV * S], BF16)   # [d, g*S + s]
            v_all = ppool.tile([128, NSC * DK], BF16)   # [s2 in chunk, sc*DK + d]

            # ---------------- Phase 1: QKV + rope + transpose ----------------
            # W-stationary: for each output-column block (24 blocks of 128:
            # 16 q-heads, 4 k-heads, 4 v-heads) accumulate psum [128, S] over
            # the 32 D-chunks with x (bf16, SBUF-resident) as moving operand.
            # Output arrives transposed ([d, s]); q/k are PE-transposed to
            # [s, d] per s-chunk for full-lane rope on DVE, then transposed
            # back into qT/kT. v is transposed once into [s2, d] layout.
            with (
                tc.tile_pool(name="p1sb", bufs=1) as p1,
                tc.tile_pool(name="p1ps", bufs=1, space="PSUM") as ps1,
            ):
                xts = []
                for dc in range(NDC):
                    xt_t = p1.tile([128, S], BF16, tag=f"xt{dc}", name=f"xt{dc}")
                    nc.sync.dma_start(out=xt_t[:], in_=xt_ext[dc])
                    xts.append(xt_t)

                # qk_sb[sc]: roped q|k in [s, d] layout, assembled then roped
                qk_sb = []
                for sc in range(NSC):
                    t = p1.tile([128, DQ + DK], BF16, tag=f"qk{sc}", name=f"qk{sc}")
                    qk_sb.append(t)

                def emit_rope(col_off, nh, sfx):
                    # rope per s-chunk on cols [col_off, col_off + nh*128)
                    for sc in range(NSC):
                        t3 = qk_sb[sc][:, col_off:col_off + nh * 128].rearrange(
                            "p (h c) -> p h c", c=128)
                        E = t3[:, :, 0:64]
                        O = t3[:, :, 64:128]
                        Cb = _bcast_like(fcc_sb[:, sc:sc + 1, :], E)
                        Db = _bcast_like(fcd_sb[:, sc:sc + 1, :], E)
                        tmps = []
                        for idx, (a, bb) in enumerate(
                                ((E, Cb), (O, Db), (E, Db), (O, Cb))):
                            t = p1.tile([128, nh, 64], BF16, tag=f"rt{idx}",
                                        bufs=2, name=f"rt{idx}_{sc}{sfx}")
                            nc.vector.tensor_tensor(out=t[:], in0=a, in1=bb,
                                                    op=mybir.AluOpType.mult)
                            tmps.append(t)
                        nc.vector.tensor_tensor(out=E, in0=tmps[0][:], in1=tmps[1][:],
                                                op=mybir.AluOpType.subtract)
                        nc.vector.tensor_tensor(out=O, in0=tmps[2][:], in1=tmps[3][:],
                                                op=mybir.AluOpType.add)

                def emit_back(col_off, n, dstT, dst_off, sfx):
                    # transpose roped [s, d] cols back into dstT [d, s] layout
                    for sc in range(NSC):
                        for hh in range(n):
                            tp = ps1.tile([128, 128], BF16, tag="tp", bufs=2,
                                          name=f"tb{sc}_{hh}{sfx}")
                            nc.tensor.transpose(
                                tp[:],
                                qk_sb[sc][:, col_off + hh * 128:
                                          col_off + (hh + 1) * 128],
                                ident_b)
                            nc.scalar.copy(
                                dstT[:, (dst_off + hh) * S + sc * 128:
                                     (dst_off + hh) * S + (sc + 1) * 128],
                                tp[:])

                # 24 column-blocks in order [k0..3, v0..3, q0..15], groups of
                # 3; the group's weight columns [128, 384] stream per D-chunk.
                # k/v first so rope-k and attention deps resolve early; rope
                # is emitted as soon as its source blocks are complete.
                for grp in range(8):
                    accs = [
                        ps1.tile([128, S], F32, tag="acc", bufs=3, name=f"acc{grp}_{b}")
                        for b in range(3)
                    ]
                    for dc in range(NDC):
                        w_t = p1.tile([128, 384], BF16, tag="w", bufs=4,
                                      name=f"w{grp}_{dc}")
                        nc.sync.dma_start(
                            out=w_t[:],
                            in_=wqkv_ext[dc, :, grp * 384:(grp + 1) * 384])
                        for b in range(3):
                            lhs = w_t[:, b * 128:(b + 1) * 128]
                            for c0 in range(0, S, 512):
                                nc.tensor.matmul(
                                    accs[b][:, c0:c0 + 512], lhs,
                                    xts[dc][:, c0:c0 + 512],
                                    start=(dc == 0), stop=(dc == NDC - 1))
                    for b in range(3):
                        blk = grp * 3 + b
                        # evict + bias (per-partition col) -> bf16 [d, s]
                        tf = p1.tile([128, S], BF16, tag="tf", bufs=3,
                                     name=f"tf{blk}")
                        nc.vector.tensor_scalar(
                            out=tf[:], in0=accs[b][:],
                            scalar1=bias_sb[:, blk:blk + 1],
                            scalar2=None, op0=mybir.AluOpType.add)
                        if blk < 4:          # k block
                            col = DQ + blk * 128
                        elif blk < 8:        # v block
                            col = None
                            g = blk - 4
                        else:                # q block
                            col = (blk - 8) * 128
                        for sc in range(NSC):
                            tp = ps1.tile([128, 128], BF16, tag="tp", bufs=2,
                                          name=f"tp{blk}_{sc}")
                            nc.tensor.transpose(
                                tp[:], tf[:, sc * 128:(sc + 1) * 128], ident_b)
                            if col is not None:
                                nc.scalar.copy(qk_sb[sc][:, col:col + 128], tp[:])
                            else:
                                nc.scalar.copy(
                                    v_all[:, sc * DK + g * 128:
                                          sc * DK + (g + 1) * 128],
                                    tp[:])
                    if grp == 1:   # k0..k3 assembled
                        emit_rope(DQ, NKV, "k")
                        emit_back(DQ, NKV, kT_all, 0, "k")
                    elif grp == 5:  # q0..q7 assembled
                        emit_rope(0, 8, "qlo")
                        emit_back(0, 8, qT_all, 0, "qlo")
                    elif grp == 7:  # q8..q15 assembled
                        emit_rope(8 * 128, 8, "qhi")
                        emit_back(8 * 128, 8, qT_all, 8, "qhi")

            # ---------------- Phase 2: attention ----------------
            # oT_all lives through phases 2+3; released manually at the end.
            p23 = tc.alloc_tile_pool(name="p23sb", bufs=1)
            oT_all = p23.tile([128, NH * S], BF16, name="oT_all")  # [d, h*S+s]
            with (
                tc.tile_pool(name="p2sb", bufs=1) as p2,
                tc.tile_pool(name="p2ps", bufs=1, space="PSUM") as ps2,
            ):
                for h in range(NH):
                    g = h // 4
                    # scoresT_j = kT_j.T @ qT (causal: s1 >= j*128), exp -> aT,
                    # then av + den accumulate immediately
                    otp = ps2.tile([128, 1024], F32, tag="ot", bufs=2, name=f"otp{h}")
                    den = ps2.tile([1, 1024], F32, tag="ot", bufs=2, name=f"den{h}")
                    for j in range(NSC):
                        if j < 4:
                            chunks = [(j * 128, 512), (512, 1024)]
                        else:
                            chunks = [(j * 128, 1024)]
                        scp = ps2.tile([128, 1024], F32, tag="sc", bufs=2,
                                       name=f"scp{h}_{j}")
                        lhs = kT_all[:, g * S + j * 128: g * S + (j + 1) * 128]
                        for (c0, c1) in chunks:
                            nc.tensor.matmul(
                                scp[:, c0:c1], lhs,
                                qT_all[:, h * S + c0: h * S + c1],
                                start=True, stop=True)
                        # causal mask on diagonal block (in-place psum add)
                        nc.vector.tensor_tensor(
                            out=scp[:, j * 128:(j + 1) * 128],
                            in0=scp[:, j * 128:(j + 1) * 128],
                            in1=maskT[:], op=mybir.AluOpType.add)
                        aT = p2.tile([128, 1024], BF16, tag="aT", bufs=3,
                                     name=f"aT{h}_{j}")
                        nc.scalar.activation(
                            aT[:, j * 128:1024], scp[:, j * 128:1024],
                            mybir.ActivationFunctionType.Exp, scale=QK_SCALE)
                        vs = v_all[:, j * DK + g * 128: j * DK + (g + 1) * 128]
                        for (c0, c1) in chunks:
                            nc.tensor.matmul(
                                otp[:, c0:c1], vs, aT[:, c0:c1],
                                start=(j == 0), stop=(j == NSC - 1),
                                skip_group_check=True)
                            nc.tensor.matmul(
                                den[:, c0:c1], ones_col[:], aT[:, c0:c1],
                                start=(j == 0), stop=(j == NSC - 1),
                                skip_group_check=True)

                    # normalize: den -> bf16 row, PE-broadcast to 128
                    # partitions, full-lane reciprocal, multiply during
                    # psum->sbuf eviction
                    drow = spool.tile([1, 1024], BF16, tag="drow", name=f"dr{h}")
                    nc.scalar.copy(drow[:], den[:])
                    rb = ps2.tile([128, 1024], F32, tag="sc", bufs=2, name=f"rb{h}")
                    for c0 in range(0, S, 512):
                        nc.tensor.matmul(rb[:, c0:c0 + 512], ones_row[:],
                                         drow[:, c0:c0 + 512],
                                         start=True, stop=True)
                    rbs = p2.tile([128, 1024], F32, tag="rbs", bufs=2, name=f"rbs{h}")
                    nc.vector.reciprocal(rbs[:], rb[:])
                    nc.vector.tensor_tensor(
                        out=oT_all[:, h * S:(h + 1) * S], in0=otp[:], in1=rbs[:],
                        op=mybir.AluOpType.mult)

            # ---------------- Phase 3: Wo ----------------
            # Groups of 4 Dout-chunks x both s-halves (8 psum banks); each
            # stationary Wo block serves both halves, Wo is streamed once.
            with (
                tc.tile_pool(name="p3sb", bufs=1) as p3,
                tc.tile_pool(name="p3ps", bufs=1, space="PSUM") as ps3,
            ):
                for mg in range(8):   # groups of 4 Dout-chunks
                    wps = [
                        ps3.tile([128, 512], F32, tag="wps", bufs=8,
                                 name=f"wps{mg}_{i}")
                        for i in range(8)  # [m0s0, m0s1, m1s0, ...]
                    ]
                    for c in range(NH):
                        wo_t = p3.tile([128, 512], BF16, tag="wo", bufs=4,
                                       name=f"wo{mg}_{c}")
                        nc.sync.dma_start(
                            out=wo_t[:],
                            in_=wo_ext[c, :, mg * 512:(mg + 1) * 512])
                        for m in range(4):
                            lhs = wo_t[:, m * 128:(m + 1) * 128]
                            for sh in range(2):
                                rhs = oT_all[:, c * S + sh * 512:
                                             c * S + sh * 512 + 512]
                                nc.tensor.matmul(
                                    wps[m * 2 + sh][:], lhs, rhs,
                                    start=(c == 0), stop=(c == NH - 1))
                    for m in range(4):
                        for sh in range(2):
                            ot_sb = p3.tile([128, 512], F32, tag="ot_sb", bufs=4,
                                            name=f"osb{mg}_{m}_{sh}")
                            nc.vector.tensor_copy(ot_sb[:], wps[m * 2 + sh][:])
                            mm = mg * 4 + m
                            nc.sync.dma_start(
                                out=out_ext[mm * 128:(mm + 1) * 128,
                                            sh * 512:(sh + 1) * 512],
                                in_=ot_sb[:])
            p23.release()

    nc.compile()
    return nc


def _evens_first_perm(nheads):
    idx = []
    for h in range(nheads):
        base = h * HD
        idx.extend(range(base, base + HD, 2))
        idx.extend(range(base + 1, base + HD, 2))
    return np.array(idx, dtype=np.int64)


def kernel(x, freqs_cis, Wq, bq, Wk, bk, Wv, bv, Wo, bo, startpos):
    global LAST_PROFILE
    x = np.asarray(x, dtype=np.float32)
    freqs_cis = np.asarray(freqs_cis, dtype=np.float32)
    Wq = np.asarray(Wq, dtype=np.float32)
    Wk = np.asarray(Wk, dtype=np.float32)
    Wv = np.asarray(Wv, dtype=np.float32)
    Wo = np.asarray(Wo, dtype=np.float32)
    bq = np.asarray(bq, dtype=np.float32)
    bk = np.asarray(bk, dtype=np.float32)
    bv = np.asarray(bv, dtype=np.float32)
    bo = np.asarray(bo, dtype=np.float32)
    assert int(startpos) == 0

    bf = lambda a: np.ascontiguousarray(a.astype(ml_dtypes.bfloat16))
    f32c = lambda a: np.ascontiguousarray(a.astype(np.float32))

    fcc = f32c(freqs_cis[:, :, 0].reshape(NSC, 128, 64).transpose(1, 0, 2))
    fcd = f32c(freqs_cis[:, :, 1].reshape(NSC, 128, 64).transpose(1, 0, 2))

    in_maps = []
    for core in range(8):
        b, g = core // 2, core % 2
        qsel = g * DQ + _evens_first_perm(NH)
        ksel = g * DK + _evens_first_perm(NKV)
        vsel = np.arange(g * DK, (g + 1) * DK)
        if core < 2:  # weight shards depend only on g; reuse for later cores
            wqkv_h = bf(np.concatenate(
                [Wk[:, ksel], Wv[:, vsel], Wq[:, qsel]], 1
            ).reshape(NDC, 128, DQ + DK + DK))
            wo_h = bf(Wo[g * DQ:(g + 1) * DQ, :].reshape(NH, 128, D))
            bqkv = np.concatenate([bk[ksel], bv[vsel], bq[qsel]])
            bqkv = f32c(bqkv.reshape(24, 128).T)  # [128, 24]: col j = block j bias
        else:
            prev = in_maps[core - 2]
            wqkv_h, wo_h, bqkv = prev["wqkv"], prev["wo"], prev["bqkv"]
        xt_h = bf(x[b].T.reshape(NDC, 128, S))
        in_maps.append({
            "xt": xt_h, "wqkv": wqkv_h, "wo": wo_h,
            "fcc": fcc, "fcd": fcd, "bqkv": bqkv,
        })

    if "nc" not in _GRAPH_CACHE:
        _GRAPH_CACHE["nc"] = _build_graph()
    nc = _GRAPH_CACHE["nc"]

    res = run_bass_kernel_spmd(
        nc, in_maps, core_ids=list(range(8)),
        trace=bool(os.environ.get("BASS_TRACE")))
    LAST_PROFILE = res

    out = np.empty((B, S, D), dtype=np.float32)
    for b in range(B):
        t = res.results[2 * b]["out"] + res.results[2 * b + 1]["out"]
        out[b] = t.T + bo[None, :]
    return out


# revision 29
# speedup vs baseline: 1.1354x; 1.0885x over previous
"""Trainium2 Bass kernel for GQA attention (B=4, S=1024, D=4096, HQ=32, HKV=8).

Sharding: 8 cores = 4 batches x 2 head-groups. Each core computes one batch
with 16 q-heads / 4 kv-heads (Wq/Wk/Wv column-sharded, Wo row-sharded). The
two head-group partial outputs per batch are summed on the host (this is the
Wo-row-shard reduction, done host-side instead of an on-device all-reduce),
then transposed (device emits out^T [Dout, S]) and bias bo added.

Device dataflow per core (SPMD, identical graph):
  Phase 1 (QKV): q[s,dq] = xT_chunk.T @ Wq_chunk (bf16, psum accumulate over
    D), evict + bias via DVE; RoPE on DVE in [s, d] layout using host-permuted
    "evens-first" head columns; PE-transpose q,k -> qT,kT [d, s] (bf16).
  Phase 2 (attention, per head): scores_i = qT_i.T @ kT (causal), diag-block
    mask added in psum, exp+rowsum fused on ScalarE (unsafe softmax - no max
    subtraction; scores are O(30) so fp32 exp is safe), normalize on DVE,
    PE-transpose attn -> attnT, av: oT += v_j.T @ attnT_j.
  Phase 3: outT = Wo_chunk.T @ oT (accumulate over dq), DMA out^T.
"""

import math
import os

import numpy as np
import ml_dtypes

import concourse.bass as bass
import concourse.mybir as mybir
import concourse.tile as tile
from concourse import bacc
from concourse.bass_utils import run_bass_kernel_spmd
from concourse.masks import make_causal_mask, make_identity

B, S, D = 4, 1024, 4096
HQ, HKV, HD = 32, 8, 128
NH = 16          # q heads per core
NKV = 4          # kv heads per core
DQ = NH * HD     # 2048
DK = NKV * HD    # 512
NDC = D // 128   # 32 D-chunks
NSC = S // 128   # 8 s-chunks
QK_SCALE = 1.0 / math.sqrt(HD)
MASK_VAL = -1e30

F32 = mybir.dt.float32
BF16 = mybir.dt.bfloat16

_GRAPH_CACHE = {}
LAST_PROFILE = None


def _bcast_like(small_ap, big_ap):
    """Broadcast small_ap (size-1 dims) against big_ap's shape."""
    a, b = bass.broadcast_tensor_aps(big_ap, small_ap)
    return b


def _build_graph():
    nc = bacc.Bacc(debug=False)

    xt_ext = nc.dram_tensor("xt", [NDC, 128, S], BF16, kind="ExternalInput")
    # packed q|k|v weight columns (host-permuted evens-first for q/k)
    wqkv_ext = nc.dram_tensor("wqkv", [NDC, 128, DQ + DK + DK], BF16,
                              kind="ExternalInput")
    wo_ext = nc.dram_tensor("wo", [NH, 128, D], BF16, kind="ExternalInput")
    fcc_ext = nc.dram_tensor("fcc", [128, NSC, 64], F32, kind="ExternalInput")
    fcd_ext = nc.dram_tensor("fcd", [128, NSC, 64], F32, kind="ExternalInput")
    # bias column tile: col j = bias for output-column-block j (q0..q15,k0..3,v0..3)
    bqkv_ext = nc.dram_tensor("bqkv", [128, 24], F32, kind="ExternalInput")
    out_ext = nc.dram_tensor("out", [D, S], F32, kind="ExternalOutput")

    with tile.TileContext(nc) as tc:
        with (
            tc.tile_pool(name="const", bufs=1) as cpool,
            tc.tile_pool(name="persist", bufs=1) as ppool,
            tc.tile_pool(name="stat", bufs=2) as spool,
        ):
            # maskT for scoresT [s2, s1]: keep s1 >= s2 (0), else -1e30
            maskT = cpool.tile([128, 128], F32)
            nc.gpsimd.memset(maskT[:], 0.0)
            nc.gpsimd.affine_select(
                out=maskT[:], in_=maskT[:], compare_op=mybir.AluOpType.is_ge,
                fill=MASK_VAL, base=0, pattern=[[1, 128]], channel_multiplier=-1)
            ident_b = cpool.tile([128, 128], BF16)
            make_identity(nc, ident_b)
            ones_col = cpool.tile([128, 1], BF16)   # den matmul lhsT
            nc.gpsimd.memset(ones_col[:], 1.0)
            ones_row = cpool.tile([1, 128], BF16)   # bcast matmul lhsT (K=1)
            nc.gpsimd.memset(ones_row[:], 1.0)
            fcc_sb = cpool.tile([128, NSC, 64], F32)
            nc.sync.dma_start(out=fcc_sb[:], in_=fcc_ext[:])
            fcd_sb = cpool.tile([128, NSC, 64], F32)
            nc.sync.dma_start(out=fcd_sb[:], in_=fcd_ext[:])
            bias_sb = cpool.tile([128, 24], F32)
            nc.sync.dma_start(out=bias_sb[:], in_=bqkv_ext[:])

            # persistent activations
            qT_all = ppool.tile([128, NH * S], BF16)    # [d, h*S + s]
            kT_all = ppool.tile([128, NKV * S], BF16)   # [d, g*S + s]
            v_all = ppool.tile([128, NSC * DK], BF16)   # [s2 in chunk, sc*DK + d]

            # ---------------- Phase 1: QKV + rope + transpose ----------------
            # W-stationary: for each output-column block (24 blocks of 128:
            # 16 q-heads, 4 k-heads, 4 v-heads) accumulate psum [128, S] over
            # the 32 D-chunks with x (bf16, SBUF-resident) as moving operand.
            # Output arrives transposed ([d, s]); q/k are PE-transposed to
            # [s, d] per s-chunk for full-lane rope on DVE, then transposed
            # back into qT/kT. v is transposed once into [s2, d] layout.
            with (
                tc.tile_pool(name="p1sb", bufs=1) as p1,
                tc.tile_pool(name="p1ps", bufs=1, space="PSUM") as ps1,
            ):
                xts = []
                for dc in range(NDC):
                    xt_t = p1.tile([128, S], BF16, tag=f"xt{dc}", name=f"xt{dc}")
                    nc.sync.dma_start(out=xt_t[:], in_=xt_ext[dc])
                    xts.append(xt_t)

                # qk_sb[sc]: roped q|k in [s, d] layout, assembled then roped
                qk_sb = []
                for sc in range(NSC):
                    t = p1.tile([128, DQ + DK], BF16, tag=f"qk{sc}", name=f"qk{sc}")
                    qk_sb.append(t)

                def emit_rope(col_off, nh, sfx):
                    # rope per s-chunk on cols [col_off, col_off + nh*128)
                    for sc in range(NSC):
                        t3 = qk_sb[sc][:, col_off:col_off + nh * 128].rearrange(
                            "p (h c) -> p h c", c=128)
                        E = t3[:, :, 0:64]
                        O = t3[:, :, 64:128]
                        Cb = _bcast_like(fcc_sb[:, sc:sc + 1, :], E)
                        Db = _bcast_like(fcd_sb[:, sc:sc + 1, :], E)
                        tmps = []
                        for idx, (a, bb) in enumerate(
                                ((E, Cb), (O, Db), (E, Db), (O, Cb))):
                            t = p1.tile([128, nh, 64], BF16, tag=f"rt{idx}",
                                        bufs=2, name=f"rt{idx}_{sc}{sfx}")
                            nc.vector.tensor_tensor(out=t[:], in0=a, in1=bb,
                                                    op=mybir.AluOpType.mult)
                            tmps.append(t)
                        nc.vector.tensor_tensor(out=E, in0=tmps[0][:], in1=tmps[1][:],
                                                op=mybir.AluOpType.subtract)
                        nc.vector.tensor_tensor(out=O, in0=tmps[2][:], in1=tmps[3][:],
                                                op=mybir.AluOpType.add)

                def emit_back(col_off, n, dstT, dst_off, sfx):
                    # transpose roped [s, d] cols back into dstT [d, s] layout
                    for sc in range(NSC):
                        for hh in range(n):
                            tp = ps1.tile([128, 128], BF16, tag="tp", bufs=2,
                                          name=f"tb{sc}_{hh}{sfx}")
                            nc.tensor.transpose(
                                tp[:],
                                qk_sb[sc][:, col_off + hh * 128:
                                          col_off + (hh + 1) * 128],
                                ident_b)
                            nc.scalar.copy(
                                dstT[:, (dst_off + hh) * S + sc * 128:
                                     (dst_off + hh) * S + (sc + 1) * 128],
                                tp[:])

                # 24 column-blocks in order [k0..3, v0..3, q0..15], groups of
                # 3; the group's weight columns [128, 384] stream per D-chunk.
                # k/v first so rope-k and attention deps resolve early; rope
                # is emitted as soon as its source blocks are complete.
                for grp in range(8):
                    accs = [
                        ps1.tile([128, S], F32, tag="acc", bufs=3, name=f"acc{grp}_{b}")
                        for b in range(3)
                    ]
                    for dc in range(NDC):
                        w_t = p1.tile([128, 384], BF16, tag="w", bufs=4,
                                      name=f"w{grp}_{dc}")
                        nc.sync.dma_start(
                            out=w_t[:],
                            in_=wqkv_ext[dc, :, grp * 384:(grp + 1) * 384])
                        for b in range(3):
                            lhs = w_t[:, b * 128:(b + 1) * 128]
                            for c0 in range(0, S, 512):
                                nc.tensor.matmul(
                                    accs[b][:, c0:c0 + 512], lhs,
                                    xts[dc][:, c0:c0 + 512],
                                    start=(dc == 0), stop=(dc == NDC - 1))
                    for b in range(3):
                        blk = grp * 3 + b
                        # evict + bias (per-partition col) -> bf16 [d, s]
                        tf = p1.tile([128, S], BF16, tag="tf", bufs=3,
                                     name=f"tf{blk}")
                        nc.vector.tensor_scalar(
                            out=tf[:], in0=accs[b][:],
                            scalar1=bias_sb[:, blk:blk + 1],
                            scalar2=None, op0=mybir.AluOpType.add)
                        if blk < 4:          # k block
                            col = DQ + blk * 128
                        elif blk < 8:        # v block
                            col = None
                            g = blk - 4
                        else:                # q block
                            col = (blk - 8) * 128
                        for sc in range(NSC):
                            tp = ps1.tile([128, 128], BF16, tag="tp", bufs=2,
                                          name=f"tp{blk}_{sc}")
                            nc.tensor.transpose(
                                tp[:], tf[:, sc * 128:(sc + 1) * 128], ident_b)
                            if col is not None:
                                nc.scalar.copy(qk_sb[sc][:, col:col + 128], tp[:])
                            else:
                                nc.scalar.copy(
                                    v_all[:, sc * DK + g * 128:
                                          sc * DK + (g + 1) * 128],
                                    tp[:])
                    if grp == 1:   # k0..k3 assembled
                        emit_rope(DQ, NKV, "k")
                        emit_back(DQ, NKV, kT_all, 0, "k")
                    elif grp == 5:  # q0..q7 assembled
                        emit_rope(0, 8, "qlo")
                        emit_back(0, 8, qT_all, 0, "qlo")
                    elif grp == 7:  # q8..q15 assembled
                        emit_rope(8 * 128, 8, "qhi")
                        emit_back(8 * 128, 8, qT_all, 8, "qhi")

            # ---------------- Phase 2: attention ----------------
            # oT_all lives through phases 2+3; released manually at the end.
            p23 = tc.alloc_tile_pool(name="p23sb", bufs=1)
            oT_all = p23.tile([128, NH * S], BF16, name="oT_all")  # [d, h*S+s]
            with (
                tc.tile_pool(name="p2sb", bufs=1) as p2,
                tc.tile_pool(name="p2ps", bufs=1, space="PSUM") as ps2,
            ):
                for h in range(NH):
                    g = h // 4
                    # scoresT_j = kT_j.T @ qT (causal: s1 >= j*128), exp -> aT,
                    # then av + den accumulate immediately
                    otp = ps2.tile([128, 1024], F32, tag="ot", bufs=2, name=f"otp{h}")
                    den = ps2.tile([1, 1024], F32, tag="ot", bufs=2, name=f"den{h}")
                    for j in range(NSC):
                        if j < 4:
                            chunks = [(j * 128, 512), (512, 1024)]
                        else:
                            chunks = [(j * 128, 1024)]
                        scp = ps2.tile([128, 1024], F32, tag="sc", bufs=2,
                                       name=f"scp{h}_{j}")
                        lhs = kT_all[:, g * S + j * 128: g * S + (j + 1) * 128]
                        for (c0, c1) in chunks:
                            nc.tensor.matmul(
                                scp[:, c0:c1], lhs,
                                qT_all[:, h * S + c0: h * S + c1],
                                start=True, stop=True)
                        # causal mask on diagonal block (in-place psum add)
                        nc.vector.tensor_tensor(
                            out=scp[:, j * 128:(j + 1) * 128],
                            in0=scp[:, j * 128:(j + 1) * 128],
                            in1=maskT[:], op=mybir.AluOpType.add)
                        aT = p2.tile([128, 1024], BF16, tag="aT", bufs=3,
                                     name=f"aT{h}_{j}")
                        nc.scalar.activation(
                            aT[:, j * 128:1024], scp[:, j * 128:1024],
                            mybir.ActivationFunctionType.Exp, scale=QK_SCALE)
                        vs = v_all[:, j * DK + g * 128: j * DK + (g + 1) * 128]
                        for (c0, c1) in chunks:
                            nc.tensor.matmul(
                                otp[:, c0:c1], vs, aT[:, c0:c1],
                                start=(j == 0), stop=(j == NSC - 1),
                                skip_group_check=True)
                            nc.tensor.matmul(
                                den[:, c0:c1], ones_col[:], aT[:, c0:c1],
                                start=(j == 0), stop=(j == NSC - 1),
                                skip_group_check=True)

                    # normalize: den -> bf16 row, PE-broadcast to 128
                    # partitions, full-lane reciprocal, multiply during
                    # psum->sbuf eviction
                    drow = spool.tile([1, 1024], BF16, tag="drow", name=f"dr{h}")
                    nc.scalar.copy(drow[:], den[:])
                    rb = ps2.tile([128, 1024], F32, tag="sc", bufs=2, name=f"rb{h}")
                    for c0 in range(0, S, 512):
                        nc.tensor.matmul(rb[:, c0:c0 + 512], ones_row[:],
                                         drow[:, c0:c0 + 512],
                                         start=True, stop=True)
                    rbs = p2.tile([128, 1024], F32, tag="rbs", bufs=2, name=f"rbs{h}")
                    nc.vector.reciprocal_approx_fast(out=rbs[:], in_=rb[:])
                    nc.vector.tensor_tensor(
                        out=oT_all[:, h * S:(h + 1) * S], in0=otp[:], in1=rbs[:],
                        op=mybir.AluOpType.mult)

            # ---------------- Phase 3: Wo ----------------
            # Groups of 4 Dout-chunks x both s-halves (8 psum banks); each
            # stationary Wo block serves both halves, Wo is streamed once.
            with (
                tc.tile_pool(name="p3sb", bufs=1) as p3,
                tc.tile_pool(name="p3ps", bufs=1, space="PSUM") as ps3,
            ):
                for mg in range(8):   # groups of 4 Dout-chunks
                    wps = [
                        ps3.tile([128, 512], F32, tag="wps", bufs=8,
                                 name=f"wps{mg}_{i}")
                        for i in range(8)  # [m0s0, m0s1, m1s0, ...]
                    ]
                    for c in range(NH):
                        wo_t = p3.tile([128, 512], BF16, tag="wo", bufs=4,
                                       name=f"wo{mg}_{c}")
                        nc.sync.dma_start(
                            out=wo_t[:],
                            in_=wo_ext[c, :, mg * 512:(mg + 1) * 512])
                        for m in range(4):
                            lhs = wo_t[:, m * 128:(m + 1) * 128]
                            for sh in range(2):
                                rhs = oT_all[:, c * S + sh * 512:
                                             c * S + sh * 512 + 512]
                                nc.tensor.matmul(
                                    wps[m * 2 + sh][:], lhs, rhs,
                                    start=(c == 0), stop=(c == NH - 1))
                    for m in range(4):
                        for sh in range(2):
                            ot_sb = p3.tile([128, 512], F32, tag="ot_sb", bufs=4,
                                            name=f"osb{mg}_{m}_{sh}")
                            nc.vector.tensor_copy(ot_sb[:], wps[m * 2 + sh][:])
                            mm = mg * 4 + m
                            nc.sync.dma_start(
                                out=out_ext[mm * 128:(mm + 1) * 128,
                                            sh * 512:(sh + 1) * 512],
                                in_=ot_sb[:])
            p23.release()

    nc.compile()
    return nc


def _evens_first_perm(nheads):
    idx = []
    for h in range(nheads):
        base = h * HD
        idx.extend(range(base, base + HD, 2))
        idx.extend(range(base + 1, base + HD, 2))
    return np.array(idx, dtype=np.int64)


def kernel(x, freqs_cis, Wq, bq, Wk, bk, Wv, bv, Wo, bo, startpos):
    global LAST_PROFILE
    x = np.asarray(x, dtype=np.float32)
    freqs_cis = np.asarray(freqs_cis, dtype=np.float32)
    Wq = np.asarray(Wq, dtype=np.float32)
    Wk = np.asarray(Wk, dtype=np.float32)
    Wv = np.asarray(Wv, dtype=np.float32)
    Wo = np.asarray(Wo, dtype=np.float32)
    bq = np.asarray(bq, dtype=np.float32)
    bk = np.asarray(bk, dtype=np.float32)
    bv = np.asarray(bv, dtype=np.float32)
    bo = np.asarray(bo, dtype=np.float32)
    assert int(startpos) == 0

    bf = lambda a: np.ascontiguousarray(a.astype(ml_dtypes.bfloat16))
    f32c = lambda a: np.ascontiguousarray(a.astype(np.float32))

    fcc = f32c(freqs_cis[:, :, 0].reshape(NSC, 128, 64).transpose(1, 0, 2))
    fcd = f32c(freqs_cis[:, :, 1].reshape(NSC, 128, 64).transpose(1, 0, 2))

    in_maps = []
    for core in range(8):
        b, g = core // 2, core % 2
        qsel = g * DQ + _evens_first_perm(NH)
        ksel = g * DK + _evens_first_perm(NKV)
        vsel = np.arange(g * DK, (g + 1) * DK)
        if core < 2:  # weight shards depend only on g; reuse for later cores
            wqkv_h = bf(np.concatenate(
                [Wk[:, ksel], Wv[:, vsel], Wq[:, qsel]], 1
            ).reshape(NDC, 128, DQ + DK + DK))
            wo_h = bf(Wo[g * DQ:(g + 1) * DQ, :].reshape(NH, 128, D))
            bqkv = np.concatenate([bk[ksel], bv[vsel], bq[qsel]])
            bqkv = f32c(bqkv.reshape(24, 128).T)  # [128, 24]: col j = block j bias
        else:
            prev = in_maps[core - 2]
            wqkv_h, wo_h, bqkv = prev["wqkv"], prev["wo"], prev["bqkv"]
        xt_h = bf(x[b].T.reshape(NDC, 128, S))
        in_maps.append({
            "xt": xt_h, "wqkv": wqkv_h, "wo": wo_h,
            "fcc": fcc, "fcd": fcd, "bqkv": bqkv,
        })

    if "nc" not in _GRAPH_CACHE:
        _GRAPH_CACHE["nc"] = _build_graph()
    nc = _GRAPH_CACHE["nc"]

    res = run_bass_kernel_spmd(
        nc, in_maps, core_ids=list(range(8)),
        trace=bool(os.environ.get("BASS_TRACE")))
    LAST_PROFILE = res

    out = np.empty((B, S, D), dtype=np.float32)
    for b in range(B):
        t = res.results[2 * b]["out"] + res.results[2 * b + 1]["out"]
        out[b] = t.T + bo[None, :]
    return out


# revision 33
# speedup vs baseline: 1.1542x; 1.0166x over previous
"""Trainium2 Bass kernel for GQA attention (B=4, S=1024, D=4096, HQ=32, HKV=8).

Sharding: 8 cores = 4 batches x 2 head-groups. Each core computes one batch
with 16 q-heads / 4 kv-heads (Wq/Wk/Wv column-sharded, Wo row-sharded). The
two head-group partial outputs per batch are summed on the host (this is the
Wo-row-shard reduction, done host-side instead of an on-device all-reduce),
then transposed (device emits out^T [Dout, S]) and bias bo added.

Device dataflow per core (SPMD, identical graph):
  Phase 1 (QKV): q[s,dq] = xT_chunk.T @ Wq_chunk (bf16, psum accumulate over
    D), evict + bias via DVE; RoPE on DVE in [s, d] layout using host-permuted
    "evens-first" head columns; PE-transpose q,k -> qT,kT [d, s] (bf16).
  Phase 2 (attention, per head): scores_i = qT_i.T @ kT (causal), diag-block
    mask added in psum, exp+rowsum fused on ScalarE (unsafe softmax - no max
    subtraction; scores are O(30) so fp32 exp is safe), normalize on DVE,
    PE-transpose attn -> attnT, av: oT += v_j.T @ attnT_j.
  Phase 3: outT = Wo_chunk.T @ oT (accumulate over dq), DMA out^T.
"""

import math
import os

import numpy as np
import ml_dtypes

import concourse.bass as bass
import concourse.mybir as mybir
import concourse.tile as tile
from concourse import bacc
from concourse.bass_utils import run_bass_kernel_spmd
from concourse.masks import make_causal_mask, make_identity

B, S, D = 4, 1024, 4096
HQ, HKV, HD = 32, 8, 128
NH = 16          # q heads per core
NKV = 4          # kv heads per core
DQ = NH * HD     # 2048
DK = NKV * HD    # 512
NDC = D // 128   # 32 D-chunks
NSC = S // 128   # 8 s-chunks
QK_SCALE = 1.0 / math.sqrt(HD)
MASK_VAL = -1e30

F32 = mybir.dt.float32
BF16 = mybir.dt.bfloat16

_GRAPH_CACHE = {}
LAST_PROFILE = None


def _bcast_like(small_ap, big_ap):
    """Broadcast small_ap (size-1 dims) against big_ap's shape."""
    a, b = bass.broadcast_tensor_aps(big_ap, small_ap)
    return b


def _build_graph():
    nc = bacc.Bacc(debug=False)

    xt_ext = nc.dram_tensor("xt", [NDC, 128, S], BF16, kind="ExternalInput")
    # packed q|k|v weight columns (host-permuted evens-first for q/k)
    wqkv_ext = nc.dram_tensor("wqkv", [NDC, 128, DQ + DK + DK], BF16,
                              kind="ExternalInput")
    wo_ext = nc.dram_tensor("wo", [NH, 128, D], BF16, kind="ExternalInput")
    fcc_ext = nc.dram_tensor("fcc", [128, NSC, 64], F32, kind="ExternalInput")
    fcd_ext = nc.dram_tensor("fcd", [128, NSC, 64], F32, kind="ExternalInput")
    # bias column tile: col j = bias for output-column-block j (q0..q15,k0..3,v0..3)
    bqkv_ext = nc.dram_tensor("bqkv", [128, 24], F32, kind="ExternalInput")
    out_ext = nc.dram_tensor("out", [D, S], F32, kind="ExternalOutput")

    with tile.TileContext(nc) as tc:
        with (
            tc.tile_pool(name="const", bufs=1) as cpool,
            tc.tile_pool(name="persist", bufs=1) as ppool,
            tc.tile_pool(name="stat", bufs=2) as spool,
        ):
            # maskT for scoresT [s2, s1]: keep s1 >= s2 (0), else -1e30
            maskT = cpool.tile([128, 128], F32)
            nc.gpsimd.memset(maskT[:], 0.0)
            nc.gpsimd.affine_select(
                out=maskT[:], in_=maskT[:], compare_op=mybir.AluOpType.is_ge,
                fill=MASK_VAL, base=0, pattern=[[1, 128]], channel_multiplier=-1)
            ident_b = cpool.tile([128, 128], BF16)
            make_identity(nc, ident_b)
            ones_col = cpool.tile([128, 1], BF16)   # den matmul lhsT
            nc.gpsimd.memset(ones_col[:], 1.0)
            ones_row = cpool.tile([1, 128], BF16)   # bcast matmul lhsT (K=1)
            nc.gpsimd.memset(ones_row[:], 1.0)
            fcc_sb = cpool.tile([128, NSC, 64], F32)
            nc.sync.dma_start(out=fcc_sb[:], in_=fcc_ext[:])
            fcd_sb = cpool.tile([128, NSC, 64], F32)
            nc.sync.dma_start(out=fcd_sb[:], in_=fcd_ext[:])
            bias_sb = cpool.tile([128, 24], F32)
            nc.sync.dma_start(out=bias_sb[:], in_=bqkv_ext[:])

            # persistent activations
            qT_all = ppool.tile([128, NH * S], BF16)    # [d, h*S + s]
            kT_all = ppool.tile([128, NKV * S], BF16)   # [d, g*S + s]
            v_all = ppool.tile([128, NSC * DK], BF16)   # [s2 in chunk, sc*DK + d]

            # ---------------- Phase 1: QKV + rope + transpose ----------------
            # W-stationary: for each output-column block (24 blocks of 128:
            # 16 q-heads, 4 k-heads, 4 v-heads) accumulate psum [128, S] over
            # the 32 D-chunks with x (bf16, SBUF-resident) as moving operand.
            # Output arrives transposed ([d, s]); q/k are PE-transposed to
            # [s, d] per s-chunk for full-lane rope on DVE, then transposed
            # back into qT/kT. v is transposed once into [s2, d] layout.
            with (
                tc.tile_pool(name="p1sb", bufs=1) as p1,
                tc.tile_pool(name="p1ps", bufs=1, space="PSUM") as ps1,
            ):
                # x tiles are DMA'd lazily inside group 0's dc loop so the
                # first group's weight DMAs aren't queued behind all of x
                xts = [
                    p1.tile([128, S], BF16, tag=f"xt{dc}", name=f"xt{dc}")
                    for dc in range(NDC)
                ]

                # qk_sb[sc]: roped q|k in [s, d] layout, assembled then roped
                qk_sb = []
                for sc in range(NSC):
                    t = p1.tile([128, DQ + DK], BF16, tag=f"qk{sc}", name=f"qk{sc}")
                    qk_sb.append(t)

                def emit_rope(col_off, nh, sfx):
                    # rope per s-chunk on cols [col_off, col_off + nh*128)
                    for sc in range(NSC):
                        t3 = qk_sb[sc][:, col_off:col_off + nh * 128].rearrange(
                            "p (h c) -> p h c", c=128)
                        E = t3[:, :, 0:64]
                        O = t3[:, :, 64:128]
                        Cb = _bcast_like(fcc_sb[:, sc:sc + 1, :], E)
                        Db = _bcast_like(fcd_sb[:, sc:sc + 1, :], E)
                        tmps = []
                        for idx, (a, bb) in enumerate(
                                ((E, Cb), (O, Db), (E, Db), (O, Cb))):
                            t = p1.tile([128, nh, 64], BF16, tag=f"rt{idx}",
                                        bufs=2, name=f"rt{idx}_{sc}{sfx}")
                            nc.vector.tensor_tensor(out=t[:], in0=a, in1=bb,
                                                    op=mybir.AluOpType.mult)
                            tmps.append(t)
                        nc.vector.tensor_tensor(out=E, in0=tmps[0][:], in1=tmps[1][:],
                                                op=mybir.AluOpType.subtract)
                        nc.vector.tensor_tensor(out=O, in0=tmps[2][:], in1=tmps[3][:],
                                                op=mybir.AluOpType.add)

                def emit_back(col_off, n, dstT, dst_off, sfx):
                    # transpose roped [s, d] cols back into dstT [d, s] layout
                    for sc in range(NSC):
                        for hh in range(n):
                            tp = ps1.tile([128, 128], BF16, tag="tp", bufs=2,
                                          name=f"tb{sc}_{hh}{sfx}")
                            nc.tensor.transpose(
                                tp[:],
                                qk_sb[sc][:, col_off + hh * 128:
                                          col_off + (hh + 1) * 128],
                                ident_b)
                            nc.scalar.copy(
                                dstT[:, (dst_off + hh) * S + sc * 128:
                                     (dst_off + hh) * S + (sc + 1) * 128],
                                tp[:])

                # 24 column-blocks in order [k0..3, v0..3, q0..15], groups of
                # 3; the group's weight columns [128, 384] stream per D-chunk.
                # k/v first so rope-k and attention deps resolve early; rope
                # is emitted as soon as its source blocks are complete.
                for grp in range(8):
                    accs = [
                        ps1.tile([128, S], F32, tag="acc", bufs=3, name=f"acc{grp}_{b}")
                        for b in range(3)
                    ]
                    for dc in range(NDC):
                        if grp == 0:
                            nc.sync.dma_start(out=xts[dc][:], in_=xt_ext[dc])
                        w_t = p1.tile([128, 384], BF16, tag="w", bufs=4,
                                      name=f"w{grp}_{dc}")
                        nc.sync.dma_start(
                            out=w_t[:],
                            in_=wqkv_ext[dc, :, grp * 384:(grp + 1) * 384])
                        for b in range(3):
                            lhs = w_t[:, b * 128:(b + 1) * 128]
                            for c0 in range(0, S, 512):
                                nc.tensor.matmul(
                                    accs[b][:, c0:c0 + 512], lhs,
                                    xts[dc][:, c0:c0 + 512],
                                    start=(dc == 0), stop=(dc == NDC - 1))
                    for b in range(3):
                        blk = grp * 3 + b
                        # evict + bias (per-partition col) -> bf16 [d, s]
                        tf = p1.tile([128, S], BF16, tag="tf", bufs=3,
                                     name=f"tf{blk}")
                        nc.vector.tensor_scalar(
                            out=tf[:], in0=accs[b][:],
                            scalar1=bias_sb[:, blk:blk + 1],
                            scalar2=None, op0=mybir.AluOpType.add)
                        if blk < 4:          # k block
                            col = DQ + blk * 128
                        elif blk < 8:        # v block
                            col = None
                            g = blk - 4
                        else:                # q block
                            col = (blk - 8) * 128
                        for sc in range(NSC):
                            tp = ps1.tile([128, 128], BF16, tag="tp", bufs=2,
                                          name=f"tp{blk}_{sc}")
                            nc.tensor.transpose(
                                tp[:], tf[:, sc * 128:(sc + 1) * 128], ident_b)
                            if col is not None:
                                nc.scalar.copy(qk_sb[sc][:, col:col + 128], tp[:])
                            else:
                                nc.scalar.copy(
                                    v_all[:, sc * DK + g * 128:
                                          sc * DK + (g + 1) * 128],
                                    tp[:])
                    if grp == 1:   # k0..k3 assembled
                        emit_rope(DQ, NKV, "k")
                        emit_back(DQ, NKV, kT_all, 0, "k")
                    elif grp == 5:  # q0..q7 assembled
                        emit_rope(0, 8, "qlo")
                        emit_back(0, 8, qT_all, 0, "qlo")
                    elif grp == 7:  # q8..q15 assembled
                        emit_rope(8 * 128, 8, "qhi")
                        emit_back(8 * 128, 8, qT_all, 8, "qhi")

            # ---------------- Phase 2: attention ----------------
            # oT_all lives through phases 2+3; released manually at the end.
            p23 = tc.alloc_tile_pool(name="p23sb", bufs=1)
            oT_all = p23.tile([128, NH * S], BF16, name="oT_all")  # [d, h*S+s]
            with (
                tc.tile_pool(name="p2sb", bufs=1) as p2,
                tc.tile_pool(name="p2ps", bufs=1, space="PSUM") as ps2,
            ):
                for h in range(NH):
                    g = h // 4
                    # scoresT_j = kT_j.T @ qT (causal: s1 >= j*128), exp -> aT,
                    # then av + den accumulate immediately
                    otp = ps2.tile([128, 1024], F32, tag="ot", bufs=2, name=f"otp{h}")
                    den = ps2.tile([1, 1024], F32, tag="ot", bufs=2, name=f"den{h}")
                    for j in range(NSC):
                        if j < 4:
                            chunks = [(j * 128, 512), (512, 1024)]
                        else:
                            chunks = [(j * 128, 1024)]
                        scp = ps2.tile([128, 1024], F32, tag="sc", bufs=2,
                                       name=f"scp{h}_{j}")
                        lhs = kT_all[:, g * S + j * 128: g * S + (j + 1) * 128]
                        for (c0, c1) in chunks:
                            nc.tensor.matmul(
                                scp[:, c0:c1], lhs,
                                qT_all[:, h * S + c0: h * S + c1],
                                start=True, stop=True)
                        # causal mask on diagonal block (in-place psum add)
                        nc.vector.tensor_tensor(
                            out=scp[:, j * 128:(j + 1) * 128],
                            in0=scp[:, j * 128:(j + 1) * 128],
                            in1=maskT[:], op=mybir.AluOpType.add)
                        aT = p2.tile([128, 1024], BF16, tag="aT", bufs=4,
                                     name=f"aT{h}_{j}")
                        nc.scalar.activation(
                            aT[:, j * 128:1024], scp[:, j * 128:1024],
                            mybir.ActivationFunctionType.Exp, scale=QK_SCALE)
                        vs = v_all[:, j * DK + g * 128: j * DK + (g + 1) * 128]
                        for (c0, c1) in chunks:
                            nc.tensor.matmul(
                                otp[:, c0:c1], vs, aT[:, c0:c1],
                                start=(j == 0), stop=(j == NSC - 1),
                                skip_group_check=True)
                            nc.tensor.matmul(
                                den[:, c0:c1], ones_col[:], aT[:, c0:c1],
                                start=(j == 0), stop=(j == NSC - 1),
                                skip_group_check=True)

                    # normalize: den -> bf16 row, PE-broadcast to 128
                    # partitions, full-lane reciprocal, multiply during
                    # psum->sbuf eviction
                    drow = spool.tile([1, 1024], BF16, tag="drow", name=f"dr{h}")
                    nc.scalar.copy(drow[:], den[:])
                    rb = ps2.tile([128, 1024], F32, tag="sc", bufs=2, name=f"rb{h}")
                    for c0 in range(0, S, 512):
                        nc.tensor.matmul(rb[:, c0:c0 + 512], ones_row[:],
                                         drow[:, c0:c0 + 512],
                                         start=True, stop=True)
                    rbs = p2.tile([128, 1024], F32, tag="rbs", bufs=2, name=f"rbs{h}")
                    nc.vector.reciprocal_approx_fast(out=rbs[:], in_=rb[:])
                    nc.vector.tensor_tensor(
                        out=oT_all[:, h * S:(h + 1) * S], in0=otp[:], in1=rbs[:],
                        op=mybir.AluOpType.mult)

            # ---------------- Phase 3: Wo ----------------
            # Groups of 4 Dout-chunks x both s-halves (8 psum banks); each
            # stationary Wo block serves both halves, Wo is streamed once.
            with (
                tc.tile_pool(name="p3sb", bufs=1) as p3,
                tc.tile_pool(name="p3ps", bufs=1, space="PSUM") as ps3,
            ):
                # 16 groups of 2 Dout-chunks x 2 s-halves (4 psum banks per
                # group, bufs=8 -> two groups in flight)
                for mg in range(16):
                    wps = [
                        ps3.tile([128, 512], F32, tag="wps", bufs=8,
                                 name=f"wps{mg}_{i}")
                        for i in range(4)  # [m0s0, m0s1, m1s0, m1s1]
                    ]
                    for c in range(NH):
                        wo_t = p3.tile([128, 256], BF16, tag="wo", bufs=6,
                                       name=f"wo{mg}_{c}")
                        nc.sync.dma_start(
                            out=wo_t[:],
                            in_=wo_ext[c, :, mg * 256:(mg + 1) * 256])
                        for m in range(2):
                            lhs = wo_t[:, m * 128:(m + 1) * 128]
                            for sh in range(2):
                                rhs = oT_all[:, c * S + sh * 512:
                                             c * S + sh * 512 + 512]
                                nc.tensor.matmul(
                                    wps[m * 2 + sh][:], lhs, rhs,
                                    start=(c == 0), stop=(c == NH - 1))
                    for m in range(2):
                        for sh in range(2):
                            ot_sb = p3.tile([128, 512], F32, tag="ot_sb", bufs=4,
                                            name=f"osb{mg}_{m}_{sh}")
                            nc.vector.tensor_copy(ot_sb[:], wps[m * 2 + sh][:])
                            mm = mg * 2 + m
                            nc.sync.dma_start(
                                out=out_ext[mm * 128:(mm + 1) * 128,
                                            sh * 512:(sh + 1) * 512],
                                in_=ot_sb[:])
            p23.release()

    nc.compile()
    return nc


def _evens_first_perm(nheads):
    idx = []
    for h in range(nheads):
        base = h * HD
        idx.extend(range(base, base + HD, 2))
        idx.extend(range(base + 1, base + HD, 2))
    return np.array(idx, dtype=np.int64)


def kernel(x, freqs_cis, Wq, bq, Wk, bk, Wv, bv, Wo, bo, startpos):
    global LAST_PROFILE
    x = np.asarray(x, dtype=np.float32)
    freqs_cis = np.asarray(freqs_cis, dtype=np.float32)
    Wq = np.asarray(Wq, dtype=np.float32)
    Wk = np.asarray(Wk, dtype=np.float32)
    Wv = np.asarray(Wv, dtype=np.float32)
    Wo = np.asarray(Wo, dtype=np.float32)
    bq = np.asarray(bq, dtype=np.float32)
    bk = np.asarray(bk, dtype=np.float32)
    bv = np.asarray(bv, dtype=np.float32)
    bo = np.asarray(bo, dtype=np.float32)
    assert int(startpos) == 0

    bf = lambda a: np.ascontiguousarray(a.astype(ml_dtypes.bfloat16))
    f32c = lambda a: np.ascontiguousarray(a.astype(np.float32))

    fcc = f32c(freqs_cis[:, :, 0].reshape(NSC, 128, 64).transpose(1, 0, 2))
    fcd = f32c(freqs_cis[:, :, 1].reshape(NSC, 128, 64).transpose(1, 0, 2))

    in_maps = []
    for core in range(8):
        b, g = core // 2, core % 2
        qsel = g * DQ + _evens_first_perm(NH)
        ksel = g * DK + _evens_first_perm(NKV)
        vsel = np.arange(g * DK, (g + 1) * DK)
        if core < 2:  # weight shards depend only on g; reuse for later cores
            wqkv_h = bf(np.concatenate(
                [Wk[:, ksel], Wv[:, vsel], Wq[:, qsel]], 1
            ).reshape(NDC, 128, DQ + DK + DK))
            wo_h = bf(Wo[g * DQ:(g + 1) * DQ, :].reshape(NH, 128, D))
            bqkv = np.concatenate([bk[ksel], bv[vsel], bq[qsel]])
            bqkv = f32c(bqkv.reshape(24, 128).T)  # [128, 24]: col j = block j bias
        else:
            prev = in_maps[core - 2]
            wqkv_h, wo_h, bqkv = prev["wqkv"], prev["wo"], prev["bqkv"]
        xt_h = bf(x[b].T.reshape(NDC, 128, S))
        in_maps.append({
            "xt": xt_h, "wqkv": wqkv_h, "wo": wo_h,
            "fcc": fcc, "fcd": fcd, "bqkv": bqkv,
        })

    if "nc" not in _GRAPH_CACHE:
        _GRAPH_CACHE["nc"] = _build_graph()
    nc = _GRAPH_CACHE["nc"]

    res = run_bass_kernel_spmd(
        nc, in_maps, core_ids=list(range(8)),
        trace=bool(os.environ.get("BASS_TRACE")))
    LAST_PROFILE = res

    out = np.empty((B, S, D), dtype=np.float32)
    for b in range(B):
        t = res.results[2 * b]["out"] + res.results[2 * b + 1]["out"]
        out[b] = t.T + bo[None, :]
    return out


# revision 40
# speedup vs baseline: 1.1796x; 1.0220x over previous
"""Trainium2 Bass kernel for GQA attention (B=4, S=1024, D=4096, HQ=32, HKV=8).

Sharding: 8 cores = 4 batches x 2 head-groups. Each core computes one batch
with 16 q-heads / 4 kv-heads (Wq/Wk/Wv column-sharded, Wo row-sharded). The
two head-group partial outputs per batch are summed on the host (this is the
Wo-row-shard reduction, done host-side instead of an on-device all-reduce),
then transposed (device emits out^T [Dout, S]) and bias bo added.

Device dataflow per core (SPMD, identical graph):
  Phase 1 (QKV): q[s,dq] = xT_chunk.T @ Wq_chunk (bf16, psum accumulate over
    D), evict + bias via DVE; RoPE on DVE in [s, d] layout using host-permuted
    "evens-first" head columns; PE-transpose q,k -> qT,kT [d, s] (bf16).
  Phase 2 (attention, per head): scores_i = qT_i.T @ kT (causal), diag-block
    mask added in psum, exp+rowsum fused on ScalarE (unsafe softmax - no max
    subtraction; scores are O(30) so fp32 exp is safe), normalize on DVE,
    PE-transpose attn -> attnT, av: oT += v_j.T @ attnT_j.
  Phase 3: outT = Wo_chunk.T @ oT (accumulate over dq), DMA out^T.
"""

import math
import os

import numpy as np
import ml_dtypes

import concourse.bass as bass
import concourse.mybir as mybir
import concourse.tile as tile
from concourse import bacc
from concourse.bass_utils import run_bass_kernel_spmd
from concourse.masks import make_causal_mask, make_identity

B, S, D = 4, 1024, 4096
HQ, HKV, HD = 32, 8, 128
NH = 16          # q heads per core
NKV = 4          # kv heads per core
DQ = NH * HD     # 2048
DK = NKV * HD    # 512
NDC = D // 128   # 32 D-chunks
NSC = S // 128   # 8 s-chunks
QK_SCALE = 1.0 / math.sqrt(HD)
MASK_VAL = -1e30

F32 = mybir.dt.float32
BF16 = mybir.dt.bfloat16

_GRAPH_CACHE = {}
LAST_PROFILE = None


def _bcast_like(small_ap, big_ap):
    """Broadcast small_ap (size-1 dims) against big_ap's shape."""
    a, b = bass.broadcast_tensor_aps(big_ap, small_ap)
    return b


def _build_graph():
    nc = bacc.Bacc(debug=False)

    xt_ext = nc.dram_tensor("xt", [NDC, 128, S], BF16, kind="ExternalInput")
    # packed q|k|v weight columns (host-permuted evens-first for q/k)
    wqkv_ext = nc.dram_tensor("wqkv", [NDC, 128, DQ + DK + DK], BF16,
                              kind="ExternalInput")
    wo_ext = nc.dram_tensor("wo", [NH, 128, D], BF16, kind="ExternalInput")
    # rope coefficient tiles for [d, s] layout (evens-first halves):
    # cd1 = [C; C], cd2 = [-D; D] with C[i, s] = fc[s, i, 0]
    cd1_ext = nc.dram_tensor("cd1", [128, S], BF16, kind="ExternalInput")
    cd2_ext = nc.dram_tensor("cd2", [128, S], BF16, kind="ExternalInput")
    # bias column tile: col j = bias for output-column-block j (q0..q15,k0..3,v0..3)
    bqkv_ext = nc.dram_tensor("bqkv", [128, 24], F32, kind="ExternalInput")
    out_ext = nc.dram_tensor("out", [D, S], F32, kind="ExternalOutput")

    with tile.TileContext(nc) as tc:
        with (
            tc.tile_pool(name="const", bufs=1) as cpool,
            tc.tile_pool(name="persist", bufs=1) as ppool,
            tc.tile_pool(name="stat", bufs=2) as spool,
        ):
            # multiplicative maskT for aT [s2, s1]: 1 where s1 >= s2 else 0
            maskT = cpool.tile([128, 128], BF16)
            nc.gpsimd.memset(maskT[:], 1.0)
            nc.gpsimd.affine_select(
                out=maskT[:], in_=maskT[:], compare_op=mybir.AluOpType.is_ge,
                fill=0.0, base=0, pattern=[[1, 128]], channel_multiplier=-1)
            ident_b = cpool.tile([128, 128], BF16)
            make_identity(nc, ident_b)
            ones_col = cpool.tile([128, 1], BF16)   # den matmul lhsT
            nc.gpsimd.memset(ones_col[:], 1.0)
            ones_row = cpool.tile([1, 128], BF16)   # bcast matmul lhsT (K=1)
            nc.gpsimd.memset(ones_row[:], 1.0)
            cd1_sb = cpool.tile([128, S], BF16)
            nc.sync.dma_start(out=cd1_sb[:], in_=cd1_ext[:])
            cd2_sb = cpool.tile([128, S], BF16)
            nc.sync.dma_start(out=cd2_sb[:], in_=cd2_ext[:])
            bias_sb = cpool.tile([128, 24], F32)
            nc.sync.dma_start(out=bias_sb[:], in_=bqkv_ext[:])

            # persistent activations
            qT_all = ppool.tile([128, NH * S], BF16)    # [d, h*S + s]
            kT_all = ppool.tile([128, NKV * S], BF16)   # [d, g*S + s]
            v_all = ppool.tile([128, NSC * DK], BF16)   # [s2 in chunk, sc*DK + d]

            # ---------------- Phase 1: QKV + rope ----------------
            # W-stationary: for each output-column block (24 blocks of 128:
            # 4 k-heads, 4 v-heads, 16 q-heads) accumulate psum [128, S] over
            # the 32 D-chunks with x (bf16, SBUF-resident) as moving operand.
            # Output arrives already transposed ([d, s]). RoPE runs directly
            # in [d, s] layout: the host permutes q/k weight columns so even
            # rope components land on partitions 0:64 and odd on 64:128; a
            # partition-swapping SBUF->SBUF DMA provides the crossed term, and
            # host-precomputed [C;C] / [-D;D] coefficient tiles feed 3 DVE
            # tensor_tensor ops that write qT/kT directly. v is PE-transposed
            # into [s2, d] layout.
            with (
                tc.tile_pool(name="p1sb", bufs=1) as p1,
                tc.tile_pool(name="p1ps", bufs=1, space="PSUM") as ps1,
            ):
                # x tiles are DMA'd lazily inside group 0's dc loop so the
                # first group's weight DMAs aren't queued behind all of x
                xts = [
                    p1.tile([128, S], BF16, tag=f"xt{dc}", name=f"xt{dc}")
                    for dc in range(NDC)
                ]

                for grp in range(8):
                    accs = [
                        ps1.tile([128, S], F32, tag="acc", bufs=3, name=f"acc{grp}_{b}")
                        for b in range(3)
                    ]
                    for dc in range(NDC):
                        if grp == 0:
                            nc.sync.dma_start(out=xts[dc][:], in_=xt_ext[dc])
                        w_t = p1.tile([128, 384], BF16, tag="w", bufs=4,
                                      name=f"w{grp}_{dc}")
                        nc.sync.dma_start(
                            out=w_t[:],
                            in_=wqkv_ext[dc, :, grp * 384:(grp + 1) * 384])
                        for b in range(3):
                            lhs = w_t[:, b * 128:(b + 1) * 128]
                            for c0 in range(0, S, 512):
                                nc.tensor.matmul(
                                    accs[b][:, c0:c0 + 512], lhs,
                                    xts[dc][:, c0:c0 + 512],
                                    start=(dc == 0), stop=(dc == NDC - 1))
                    for b in range(3):
                        blk = grp * 3 + b
                        # evict + bias (per-partition col) -> bf16 [d, s]
                        tf = p1.tile([128, S], BF16, tag="tf", bufs=3,
                                     name=f"tf{blk}")
                        nc.vector.tensor_scalar(
                            out=tf[:], in0=accs[b][:],
                            scalar1=bias_sb[:, blk:blk + 1],
                            scalar2=None, op0=mybir.AluOpType.add)
                        if 4 <= blk < 8:
                            # v block: PE-transpose into [s2, d] layout
                            g = blk - 4
                            for sc in range(NSC):
                                tp = ps1.tile([128, 128], BF16, tag="tp", bufs=2,
                                              name=f"tp{blk}_{sc}")
                                nc.tensor.transpose(
                                    tp[:], tf[:, sc * 128:(sc + 1) * 128], ident_b)
                                nc.scalar.copy(
                                    v_all[:, sc * DK + g * 128:
                                          sc * DK + (g + 1) * 128],
                                    tp[:])
                        else:
                            # q/k block: rope in [d, s] layout
                            if blk < 4:
                                dstT, idx = kT_all, blk
                            else:
                                dstT, idx = qT_all, blk - 8
                            tfs = p1.tile([128, S], BF16, tag="tfs", bufs=3,
                                          name=f"tfs{blk}")
                            nc.sync.dma_start(out=tfs[0:64, :], in_=tf[64:128, :])
                            nc.sync.dma_start(out=tfs[64:128, :], in_=tf[0:64, :])
                            t1 = p1.tile([128, S], BF16, tag="rt0", bufs=3,
                                         name=f"rt0_{blk}")
                            nc.vector.tensor_tensor(
                                out=t1[:], in0=tf[:], in1=cd1_sb[:],
                                op=mybir.AluOpType.mult)
                            t2 = p1.tile([128, S], BF16, tag="rt1", bufs=3,
                                         name=f"rt1_{blk}")
                            nc.vector.tensor_tensor(
                                out=t2[:], in0=tfs[:], in1=cd2_sb[:],
                                op=mybir.AluOpType.mult)
                            nc.vector.tensor_tensor(
                                out=dstT[:, idx * S:(idx + 1) * S],
                                in0=t1[:], in1=t2[:], op=mybir.AluOpType.add)

            # ---------------- Phase 2: attention ----------------
            # oT_all lives through phases 2+3; released manually at the end.
            p23 = tc.alloc_tile_pool(name="p23sb", bufs=1)
            oT_all = p23.tile([128, NH * S], BF16, name="oT_all")  # [d, h*S+s]
            with (
                tc.tile_pool(name="p2sb", bufs=1) as p2,
                tc.tile_pool(name="p2ps", bufs=1, space="PSUM") as ps2,
            ):
                for h in range(NH):
                    g = h // 4
                    # scoresT_j = kT_j.T @ qT (causal: s1 >= j*128), exp -> aT,
                    # then av + den accumulate immediately
                    otp = ps2.tile([128, 1024], F32, tag="ot", bufs=2, name=f"otp{h}")
                    den = ps2.tile([1, 1024], F32, tag="ot", bufs=2, name=f"den{h}")
                    for j in range(NSC):
                        if j < 4:
                            chunks = [(j * 128, 512), (512, 1024)]
                        else:
                            chunks = [(j * 128, 1024)]
                        scp = ps2.tile([128, 1024], F32, tag="sc", bufs=2,
                                       name=f"scp{h}_{j}")
                        lhs = kT_all[:, g * S + j * 128: g * S + (j + 1) * 128]
                        for (c0, c1) in chunks:
                            nc.tensor.matmul(
                                scp[:, c0:c1], lhs,
                                qT_all[:, h * S + c0: h * S + c1],
                                start=True, stop=True)
                        aT = p2.tile([128, 1024], BF16, tag="aT", bufs=4,
                                     name=f"aT{h}_{j}")
                        nc.scalar.activation(
                            aT[:, j * 128:1024], scp[:, j * 128:1024],
                            mybir.ActivationFunctionType.Exp, scale=QK_SCALE)
                        # causal mask on diagonal block (multiplicative, bf16)
                        nc.vector.tensor_tensor(
                            out=aT[:, j * 128:(j + 1) * 128],
                            in0=aT[:, j * 128:(j + 1) * 128],
                            in1=maskT[:], op=mybir.AluOpType.mult)
                        vs = v_all[:, j * DK + g * 128: j * DK + (g + 1) * 128]
                        for (c0, c1) in chunks:
                            nc.tensor.matmul(
                                otp[:, c0:c1], vs, aT[:, c0:c1],
                                start=(j == 0), stop=(j == NSC - 1),
                                skip_group_check=True)
                            nc.tensor.matmul(
                                den[:, c0:c1], ones_col[:], aT[:, c0:c1],
                                start=(j == 0), stop=(j == NSC - 1),
                                skip_group_check=True)

                    # normalize: den -> bf16 row, PE-broadcast to 128
                    # partitions, full-lane reciprocal, multiply during
                    # psum->sbuf eviction
                    drow = spool.tile([1, 1024], BF16, tag="drow", name=f"dr{h}")
                    nc.scalar.copy(drow[:], den[:])
                    rb = ps2.tile([128, 1024], F32, tag="sc", bufs=2, name=f"rb{h}")
                    for c0 in range(0, S, 512):
                        nc.tensor.matmul(rb[:, c0:c0 + 512], ones_row[:],
                                         drow[:, c0:c0 + 512],
                                         start=True, stop=True)
                    rbs = p2.tile([128, 1024], F32, tag="rbs", bufs=2, name=f"rbs{h}")
                    nc.vector.reciprocal_approx_fast(out=rbs[:], in_=rb[:])
                    nc.vector.tensor_tensor(
                        out=oT_all[:, h * S:(h + 1) * S], in0=otp[:], in1=rbs[:],
                        op=mybir.AluOpType.mult)

            # ---------------- Phase 3: Wo ----------------
            # Groups of 4 Dout-chunks x both s-halves (8 psum banks); each
            # stationary Wo block serves both halves, Wo is streamed once.
            with (
                tc.tile_pool(name="p3sb", bufs=1) as p3,
                tc.tile_pool(name="p3ps", bufs=1, space="PSUM") as ps3,
            ):
                # 16 groups of 2 Dout-chunks x 2 s-halves (4 psum banks per
                # group, bufs=8 -> two groups in flight)
                for mg in range(16):
                    wps = [
                        ps3.tile([128, 512], F32, tag="wps", bufs=8,
                                 name=f"wps{mg}_{i}")
                        for i in range(4)  # [m0s0, m0s1, m1s0, m1s1]
                    ]
                    for c in range(NH):
                        wo_t = p3.tile([128, 256], BF16, tag="wo", bufs=6,
                                       name=f"wo{mg}_{c}")
                        nc.sync.dma_start(
                            out=wo_t[:],
                            in_=wo_ext[c, :, mg * 256:(mg + 1) * 256])
                        for m in range(2):
                            lhs = wo_t[:, m * 128:(m + 1) * 128]
                            for sh in range(2):
                                rhs = oT_all[:, c * S + sh * 512:
                                             c * S + sh * 512 + 512]
                                nc.tensor.matmul(
                                    wps[m * 2 + sh][:], lhs, rhs,
                                    start=(c == 0), stop=(c == NH - 1))
                    for m in range(2):
                        for sh in range(2):
                            ot_sb = p3.tile([128, 512], F32, tag="ot_sb", bufs=4,
                                            name=f"osb{mg}_{m}_{sh}")
                            nc.vector.tensor_copy(ot_sb[:], wps[m * 2 + sh][:])
                            mm = mg * 2 + m
                            nc.sync.dma_start(
                                out=out_ext[mm * 128:(mm + 1) * 128,
                                            sh * 512:(sh + 1) * 512],
                                in_=ot_sb[:])
            p23.release()

    nc.compile()
    return nc


def _evens_first_perm(nheads):
    idx = []
    for h in range(nheads):
        base = h * HD
        idx.extend(range(base, base + HD, 2))
        idx.extend(range(base + 1, base + HD, 2))
    return np.array(idx, dtype=np.int64)


def kernel(x, freqs_cis, Wq, bq, Wk, bk, Wv, bv, Wo, bo, startpos):
    global LAST_PROFILE
    x = np.asarray(x, dtype=np.float32)
    freqs_cis = np.asarray(freqs_cis, dtype=np.float32)
    Wq = np.asarray(Wq, dtype=np.float32)
    Wk = np.asarray(Wk, dtype=np.float32)
    Wv = np.asarray(Wv, dtype=np.float32)
    Wo = np.asarray(Wo, dtype=np.float32)
    bq = np.asarray(bq, dtype=np.float32)
    bk = np.asarray(bk, dtype=np.float32)
    bv = np.asarray(bv, dtype=np.float32)
    bo = np.asarray(bo, dtype=np.float32)
    assert int(startpos) == 0

    bf = lambda a: np.ascontiguousarray(a.astype(ml_dtypes.bfloat16))
    f32c = lambda a: np.ascontiguousarray(a.astype(np.float32))

    # rope coefficients in [d, s] layout: C64[i, s] = fc[s, i, 0]
    C64 = freqs_cis[:, :, 0].T          # [64, S]
    D64 = freqs_cis[:, :, 1].T
    cd1 = bf(np.vstack([C64, C64]))     # coeff on tf   (rows 0:64 = E, 64: = O)
    cd2 = bf(np.vstack([-D64, D64]))    # coeff on swapped tf

    in_maps = []
    for core in range(8):
        b, g = core // 2, core % 2
        qsel = g * DQ + _evens_first_perm(NH)
        ksel = g * DK + _evens_first_perm(NKV)
        vsel = np.arange(g * DK, (g + 1) * DK)
        if core < 2:  # weight shards depend only on g; reuse for later cores
            wqkv_h = bf(np.concatenate(
                [Wk[:, ksel], Wv[:, vsel], Wq[:, qsel]], 1
            ).reshape(NDC, 128, DQ + DK + DK))
            wo_h = bf(Wo[g * DQ:(g + 1) * DQ, :].reshape(NH, 128, D))
            bqkv = np.concatenate([bk[ksel], bv[vsel], bq[qsel]])
            bqkv = f32c(bqkv.reshape(24, 128).T)  # [128, 24]: col j = block j bias
        else:
            prev = in_maps[core - 2]
            wqkv_h, wo_h, bqkv = prev["wqkv"], prev["wo"], prev["bqkv"]
        xt_h = bf(x[b].T.reshape(NDC, 128, S))
        in_maps.append({
            "xt": xt_h, "wqkv": wqkv_h, "wo": wo_h,
            "cd1": cd1, "cd2": cd2, "bqkv": bqkv,
        })

    if "nc" not in _GRAPH_CACHE:
        _GRAPH_CACHE["nc"] = _build_graph()
    nc = _GRAPH_CACHE["nc"]

    res = run_bass_kernel_spmd(
        nc, in_maps, core_ids=list(range(8)),
        trace=bool(os.environ.get("BASS_TRACE")))
    LAST_PROFILE = res

    out = np.empty((B, S, D), dtype=np.float32)
    for b in range(B):
        t = res.results[2 * b]["out"] + res.results[2 * b + 1]["out"]
        out[b] = t.T + bo[None, :]
    return out


# revision 42
# speedup vs baseline: 1.2460x; 1.0563x over previous
"""Trainium2 Bass kernel for GQA attention (B=4, S=1024, D=4096, HQ=32, HKV=8).

Sharding: 8 cores = 4 batches x 2 head-groups. Each core computes one batch
with 16 q-heads / 4 kv-heads (Wq/Wk/Wv column-sharded, Wo row-sharded). The
two head-group partial outputs per batch are summed on the host (this is the
Wo-row-shard reduction, done host-side instead of an on-device all-reduce),
then transposed (device emits out^T [Dout, S]) and bias bo added.

Device dataflow per core (SPMD, identical graph):
  Phase 1 (QKV): q[s,dq] = xT_chunk.T @ Wq_chunk (bf16, psum accumulate over
    D), evict + bias via DVE; RoPE on DVE in [s, d] layout using host-permuted
    "evens-first" head columns; PE-transpose q,k -> qT,kT [d, s] (bf16).
  Phase 2 (attention, per head): scores_i = qT_i.T @ kT (causal), diag-block
    mask added in psum, exp+rowsum fused on ScalarE (unsafe softmax - no max
    subtraction; scores are O(30) so fp32 exp is safe), normalize on DVE,
    PE-transpose attn -> attnT, av: oT += v_j.T @ attnT_j.
  Phase 3: outT = Wo_chunk.T @ oT (accumulate over dq), DMA out^T.
"""

import math
import os

import numpy as np
import ml_dtypes

import concourse.bass as bass
import concourse.mybir as mybir
import concourse.tile as tile
from concourse import bacc
from concourse.bass_utils import run_bass_kernel_spmd
from concourse.masks import make_causal_mask, make_identity

B, S, D = 4, 1024, 4096
HQ, HKV, HD = 32, 8, 128
NH = 16          # q heads per core
NKV = 4          # kv heads per core
DQ = NH * HD     # 2048
DK = NKV * HD    # 512
NDC = D // 128   # 32 D-chunks
NSC = S // 128   # 8 s-chunks
QK_SCALE = 1.0 / math.sqrt(HD)
MASK_VAL = -1e30

F32 = mybir.dt.float32
BF16 = mybir.dt.bfloat16

_GRAPH_CACHE = {}
LAST_PROFILE = None


def _bcast_like(small_ap, big_ap):
    """Broadcast small_ap (size-1 dims) against big_ap's shape."""
    a, b = bass.broadcast_tensor_aps(big_ap, small_ap)
    return b


def _build_graph():
    nc = bacc.Bacc(debug=False)

    xt_ext = nc.dram_tensor("xt", [NDC, 128, S], BF16, kind="ExternalInput")
    # packed q|k|v weight columns (host-permuted evens-first for q/k)
    wqkv_ext = nc.dram_tensor("wqkv", [NDC, 128, DQ + DK + DK], BF16,
                              kind="ExternalInput")
    wo_ext = nc.dram_tensor("wo", [NH, 128, D], BF16, kind="ExternalInput")
    # rope coefficient tiles for [d, s] layout (evens-first halves):
    # cd1 = [C; C], cd2 = [-D; D] with C[i, s] = fc[s, i, 0]
    cd1_ext = nc.dram_tensor("cd1", [128, S], BF16, kind="ExternalInput")
    cd2_ext = nc.dram_tensor("cd2", [128, S], BF16, kind="ExternalInput")
    # bias column tile: col j = bias for output-column-block j (q0..q15,k0..3,v0..3)
    bqkv_ext = nc.dram_tensor("bqkv", [128, 24], F32, kind="ExternalInput")
    out_ext = nc.dram_tensor("out", [D, S], F32, kind="ExternalOutput")

    with tile.TileContext(nc) as tc:
        with (
            tc.tile_pool(name="const", bufs=1) as cpool,
            tc.tile_pool(name="persist", bufs=1) as ppool,
            tc.tile_pool(name="stat", bufs=2) as spool,
        ):
            # multiplicative maskT for aT [s2, s1]: 1 where s1 >= s2 else 0
            maskT = cpool.tile([128, 128], BF16)
            nc.gpsimd.memset(maskT[:], 1.0)
            nc.gpsimd.affine_select(
                out=maskT[:], in_=maskT[:], compare_op=mybir.AluOpType.is_ge,
                fill=0.0, base=0, pattern=[[1, 128]], channel_multiplier=-1)
            ident_b = cpool.tile([128, 128], BF16)
            make_identity(nc, ident_b)
            ones_col = cpool.tile([128, 1], BF16)   # den matmul lhsT
            nc.gpsimd.memset(ones_col[:], 1.0)
            ones_row = cpool.tile([1, 128], BF16)   # bcast matmul lhsT (K=1)
            nc.gpsimd.memset(ones_row[:], 1.0)
            cd1_sb = cpool.tile([128, S], BF16)
            nc.sync.dma_start(out=cd1_sb[:], in_=cd1_ext[:])
            cd2_sb = cpool.tile([128, S], BF16)
            nc.sync.dma_start(out=cd2_sb[:], in_=cd2_ext[:])
            bias_sb = cpool.tile([128, 24], F32)
            nc.sync.dma_start(out=bias_sb[:], in_=bqkv_ext[:])

            # persistent activations
            qT_all = ppool.tile([128, NH * S], BF16)    # [d, h*S + s]
            kT_all = ppool.tile([128, NKV * S], BF16)   # [d, g*S + s]
            v_all = ppool.tile([128, NSC * DK], BF16)   # [s2 in chunk, sc*DK + d]

            # ---------------- Phase 1: QKV + rope ----------------
            # W-stationary: for each output-column block (24 blocks of 128:
            # 4 k-heads, 4 v-heads, 16 q-heads) accumulate psum [128, S] over
            # the 32 D-chunks with x (bf16, SBUF-resident) as moving operand.
            # Output arrives already transposed ([d, s]). RoPE runs directly
            # in [d, s] layout: the host permutes q/k weight columns so even
            # rope components land on partitions 0:64 and odd on 64:128; a
            # partition-swapping SBUF->SBUF DMA provides the crossed term, and
            # host-precomputed [C;C] / [-D;D] coefficient tiles feed 3 DVE
            # tensor_tensor ops that write qT/kT directly. v is PE-transposed
            # into [s2, d] layout.
            with (
                tc.tile_pool(name="p1sb", bufs=1) as p1,
                tc.tile_pool(name="p1ps", bufs=1, space="PSUM") as ps1,
            ):
                # x tiles are DMA'd lazily inside group 0's dc loop so the
                # first group's weight DMAs aren't queued behind all of x
                xts = [
                    p1.tile([128, S], BF16, tag=f"xt{dc}", name=f"xt{dc}")
                    for dc in range(NDC)
                ]

                for grp in range(8):
                    accs = [
                        ps1.tile([128, S], F32, tag="acc", bufs=3, name=f"acc{grp}_{b}")
                        for b in range(3)
                    ]
                    for dc in range(NDC):
                        if grp == 0:
                            nc.sync.dma_start(out=xts[dc][:], in_=xt_ext[dc])
                        w_t = p1.tile([128, 384], BF16, tag="w", bufs=4,
                                      name=f"w{grp}_{dc}")
                        nc.sync.dma_start(
                            out=w_t[:],
                            in_=wqkv_ext[dc, :, grp * 384:(grp + 1) * 384])
                        for b in range(3):
                            lhs = w_t[:, b * 128:(b + 1) * 128]
                            for c0 in range(0, S, 512):
                                nc.tensor.matmul(
                                    accs[b][:, c0:c0 + 512], lhs,
                                    xts[dc][:, c0:c0 + 512],
                                    start=(dc == 0), stop=(dc == NDC - 1))
                    for b in range(3):
                        blk = grp * 3 + b
                        # evict + bias (per-partition col) -> bf16 [d, s]
                        tf = p1.tile([128, S], BF16, tag="tf", bufs=3,
                                     name=f"tf{blk}")
                        nc.vector.tensor_scalar(
                            out=tf[:], in0=accs[b][:],
                            scalar1=bias_sb[:, blk:blk + 1],
                            scalar2=None, op0=mybir.AluOpType.add)
                        if 4 <= blk < 8:
                            # v block: PE-transpose into [s2, d] layout
                            g = blk - 4
                            for sc in range(NSC):
                                tp = ps1.tile([128, 128], BF16, tag="tp", bufs=2,
                                              name=f"tp{blk}_{sc}")
                                nc.tensor.transpose(
                                    tp[:], tf[:, sc * 128:(sc + 1) * 128], ident_b)
                                nc.scalar.copy(
                                    v_all[:, sc * DK + g * 128:
                                          sc * DK + (g + 1) * 128],
                                    tp[:])
                        else:
                            # q/k block: rope in [d, s] layout
                            if blk < 4:
                                dstT, idx = kT_all, blk
                            else:
                                dstT, idx = qT_all, blk - 8
                            tfs = p1.tile([128, S], BF16, tag="tfs", bufs=3,
                                          name=f"tfs{blk}")
                            nc.gpsimd.dma_start(out=tfs[0:64, :], in_=tf[64:128, :])
                            nc.gpsimd.dma_start(out=tfs[64:128, :], in_=tf[0:64, :])
                            t1 = p1.tile([128, S], BF16, tag="rt0", bufs=3,
                                         name=f"rt0_{blk}")
                            nc.vector.tensor_tensor(
                                out=t1[:], in0=tf[:], in1=cd1_sb[:],
                                op=mybir.AluOpType.mult)
                            t2 = p1.tile([128, S], BF16, tag="rt1", bufs=3,
                                         name=f"rt1_{blk}")
                            nc.vector.tensor_tensor(
                                out=t2[:], in0=tfs[:], in1=cd2_sb[:],
                                op=mybir.AluOpType.mult)
                            nc.vector.tensor_tensor(
                                out=dstT[:, idx * S:(idx + 1) * S],
                                in0=t1[:], in1=t2[:], op=mybir.AluOpType.add)

            # ---------------- Phase 2: attention ----------------
            # oT_all lives through phases 2+3; released manually at the end.
            p23 = tc.alloc_tile_pool(name="p23sb", bufs=1)
            oT_all = p23.tile([128, NH * S], BF16, name="oT_all")  # [d, h*S+s]
            with (
                tc.tile_pool(name="p2sb", bufs=1) as p2,
                tc.tile_pool(name="p2ps", bufs=1, space="PSUM") as ps2,
            ):
                for h in range(NH):
                    g = h // 4
                    # scoresT_j = kT_j.T @ qT (causal: s1 >= j*128), exp -> aT,
                    # then av + den accumulate immediately
                    otp = ps2.tile([128, 1024], F32, tag="ot", bufs=2, name=f"otp{h}")
                    den = ps2.tile([1, 1024], F32, tag="ot", bufs=2, name=f"den{h}")
                    for j in range(NSC):
                        if j < 4:
                            chunks = [(j * 128, 512), (512, 1024)]
                        else:
                            chunks = [(j * 128, 1024)]
                        scp = ps2.tile([128, 1024], F32, tag="sc", bufs=2,
                                       name=f"scp{h}_{j}")
                        lhs = kT_all[:, g * S + j * 128: g * S + (j + 1) * 128]
                        for (c0, c1) in chunks:
                            nc.tensor.matmul(
                                scp[:, c0:c1], lhs,
                                qT_all[:, h * S + c0: h * S + c1],
                                start=True, stop=True)
                        aT = p2.tile([128, 1024], BF16, tag="aT", bufs=4,
                                     name=f"aT{h}_{j}")
                        nc.scalar.activation(
                            aT[:, j * 128:1024], scp[:, j * 128:1024],
                            mybir.ActivationFunctionType.Exp, scale=QK_SCALE)
                        # causal mask on diagonal block (multiplicative, bf16)
                        nc.vector.tensor_tensor(
                            out=aT[:, j * 128:(j + 1) * 128],
                            in0=aT[:, j * 128:(j + 1) * 128],
                            in1=maskT[:], op=mybir.AluOpType.mult)
                        vs = v_all[:, j * DK + g * 128: j * DK + (g + 1) * 128]
                        for (c0, c1) in chunks:
                            nc.tensor.matmul(
                                otp[:, c0:c1], vs, aT[:, c0:c1],
                                start=(j == 0), stop=(j == NSC - 1),
                                skip_group_check=True)
                            nc.tensor.matmul(
                                den[:, c0:c1], ones_col[:], aT[:, c0:c1],
                                start=(j == 0), stop=(j == NSC - 1),
                                skip_group_check=True)

                    # normalize: den -> bf16 row, PE-broadcast to 128
                    # partitions, full-lane reciprocal, multiply during
                    # psum->sbuf eviction
                    drow = spool.tile([1, 1024], BF16, tag="drow", name=f"dr{h}")
                    nc.scalar.copy(drow[:], den[:])
                    rb = ps2.tile([128, 1024], F32, tag="sc", bufs=2, name=f"rb{h}")
                    for c0 in range(0, S, 512):
                        nc.tensor.matmul(rb[:, c0:c0 + 512], ones_row[:],
                                         drow[:, c0:c0 + 512],
                                         start=True, stop=True)
                    rbs = p2.tile([128, 1024], F32, tag="rbs", bufs=2, name=f"rbs{h}")
                    nc.vector.reciprocal_approx_fast(out=rbs[:], in_=rb[:])
                    nc.vector.tensor_tensor(
                        out=oT_all[:, h * S:(h + 1) * S], in0=otp[:], in1=rbs[:],
                        op=mybir.AluOpType.mult)

            # ---------------- Phase 3: Wo ----------------
            # Groups of 4 Dout-chunks x both s-halves (8 psum banks); each
            # stationary Wo block serves both halves, Wo is streamed once.
            with (
                tc.tile_pool(name="p3sb", bufs=1) as p3,
                tc.tile_pool(name="p3ps", bufs=1, space="PSUM") as ps3,
            ):
                # 16 groups of 2 Dout-chunks x 2 s-halves (4 psum banks per
                # group, bufs=8 -> two groups in flight)
                for mg in range(16):
                    wps = [
                        ps3.tile([128, 512], F32, tag="wps", bufs=8,
                                 name=f"wps{mg}_{i}")
                        for i in range(4)  # [m0s0, m0s1, m1s0, m1s1]
                    ]
                    for c in range(NH):
                        wo_t = p3.tile([128, 256], BF16, tag="wo", bufs=6,
                                       name=f"wo{mg}_{c}")
                        nc.gpsimd.dma_start(
                            out=wo_t[:],
                            in_=wo_ext[c, :, mg * 256:(mg + 1) * 256])
                        for m in range(2):
                            lhs = wo_t[:, m * 128:(m + 1) * 128]
                            for sh in range(2):
                                rhs = oT_all[:, c * S + sh * 512:
                                             c * S + sh * 512 + 512]
                                nc.tensor.matmul(
                                    wps[m * 2 + sh][:], lhs, rhs,
                                    start=(c == 0), stop=(c == NH - 1))
                    for m in range(2):
                        for sh in range(2):
                            ot_sb = p3.tile([128, 512], F32, tag="ot_sb", bufs=4,
                                            name=f"osb{mg}_{m}_{sh}")
                            nc.vector.tensor_copy(ot_sb[:], wps[m * 2 + sh][:])
                            mm = mg * 2 + m
                            nc.sync.dma_start(
                                out=out_ext[mm * 128:(mm + 1) * 128,
                                            sh * 512:(sh + 1) * 512],
                                in_=ot_sb[:])
            p23.release()

    nc.compile()
    return nc


def _evens_first_perm(nheads):
    idx = []
    for h in range(nheads):
        base = h * HD
        idx.extend(range(base, base + HD, 2))
        idx.extend(range(base + 1, base + HD, 2))
    return np.array(idx, dtype=np.int64)


def kernel(x, freqs_cis, Wq, bq, Wk, bk, Wv, bv, Wo, bo, startpos):
    global LAST_PROFILE
    x = np.asarray(x, dtype=np.float32)
    freqs_cis = np.asarray(freqs_cis, dtype=np.float32)
    Wq = np.asarray(Wq, dtype=np.float32)
    Wk = np.asarray(Wk, dtype=np.float32)
    Wv = np.asarray(Wv, dtype=np.float32)
    Wo = np.asarray(Wo, dtype=np.float32)
    bq = np.asarray(bq, dtype=np.float32)
    bk = np.asarray(bk, dtype=np.float32)
    bv = np.asarray(bv, dtype=np.float32)
    bo = np.asarray(bo, dtype=np.float32)
    assert int(startpos) == 0

    bf = lambda a: np.ascontiguousarray(a.astype(ml_dtypes.bfloat16))
    f32c = lambda a: np.ascontiguousarray(a.astype(np.float32))

    # rope coefficients in [d, s] layout: C64[i, s] = fc[s, i, 0]
    C64 = freqs_cis[:, :, 0].T          # [64, S]
    D64 = freqs_cis[:, :, 1].T
    cd1 = bf(np.vstack([C64, C64]))     # coeff on tf   (rows 0:64 = E, 64: = O)
    cd2 = bf(np.vstack([-D64, D64]))    # coeff on swapped tf

    in_maps = []
    for core in range(8):
        b, g = core // 2, core % 2
        qsel = g * DQ + _evens_first_perm(NH)
        ksel = g * DK + _evens_first_perm(NKV)
        vsel = np.arange(g * DK, (g + 1) * DK)
        if core < 2:  # weight shards depend only on g; reuse for later cores
            wqkv_h = bf(np.concatenate(
                [Wk[:, ksel], Wv[:, vsel], Wq[:, qsel]], 1
            ).reshape(NDC, 128, DQ + DK + DK))
            wo_h = bf(Wo[g * DQ:(g + 1) * DQ, :].reshape(NH, 128, D))
            bqkv = np.concatenate([bk[ksel], bv[vsel], bq[qsel]])
            bqkv = f32c(bqkv.reshape(24, 128).T)  # [128, 24]: col j = block j bias
        else:
            prev = in_maps[core - 2]
            wqkv_h, wo_h, bqkv = prev["wqkv"], prev["wo"], prev["bqkv"]
        xt_h = bf(x[b].T.reshape(NDC, 128, S))
        in_maps.append({
            "xt": xt_h, "wqkv": wqkv_h, "wo": wo_h,
            "cd1": cd1, "cd2": cd2, "bqkv": bqkv,
        })

    if "nc" not in _GRAPH_CACHE:
        _GRAPH_CACHE["nc"] = _build_graph()
    nc = _GRAPH_CACHE["nc"]

    res = run_bass_kernel_spmd(
        nc, in_maps, core_ids=list(range(8)),
        trace=bool(os.environ.get("BASS_TRACE")))
    LAST_PROFILE = res

    out = np.empty((B, S, D), dtype=np.float32)
    for b in range(B):
        t = res.results[2 * b]["out"] + res.results[2 * b + 1]["out"]
        out[b] = t.T + bo[None, :]
    return out


# revision 44
# speedup vs baseline: 1.3180x; 1.0578x over previous
"""Trainium2 Bass kernel for GQA attention (B=4, S=1024, D=4096, HQ=32, HKV=8).

Sharding: 8 cores = 4 batches x 2 head-groups. Each core computes one batch
with 16 q-heads / 4 kv-heads (Wq/Wk/Wv column-sharded, Wo row-sharded). The
two head-group partial outputs per batch are summed on the host (this is the
Wo-row-shard reduction, done host-side instead of an on-device all-reduce),
then transposed (device emits out^T [Dout, S]) and bias bo added.

Device dataflow per core (SPMD, identical graph):
  Phase 1 (QKV): q[s,dq] = xT_chunk.T @ Wq_chunk (bf16, psum accumulate over
    D), evict + bias via DVE; RoPE on DVE in [s, d] layout using host-permuted
    "evens-first" head columns; PE-transpose q,k -> qT,kT [d, s] (bf16).
  Phase 2 (attention, per head): scores_i = qT_i.T @ kT (causal), diag-block
    mask added in psum, exp+rowsum fused on ScalarE (unsafe softmax - no max
    subtraction; scores are O(30) so fp32 exp is safe), normalize on DVE,
    PE-transpose attn -> attnT, av: oT += v_j.T @ attnT_j.
  Phase 3: outT = Wo_chunk.T @ oT (accumulate over dq), DMA out^T.
"""

import math
import os

import numpy as np
import ml_dtypes

import concourse.bass as bass
import concourse.mybir as mybir
import concourse.tile as tile
from concourse import bacc
from concourse.bass_utils import run_bass_kernel_spmd
from concourse.masks import make_causal_mask, make_identity

B, S, D = 4, 1024, 4096
HQ, HKV, HD = 32, 8, 128
NH = 16          # q heads per core
NKV = 4          # kv heads per core
DQ = NH * HD     # 2048
DK = NKV * HD    # 512
NDC = D // 128   # 32 D-chunks
NSC = S // 128   # 8 s-chunks
QK_SCALE = 1.0 / math.sqrt(HD)
MASK_VAL = -1e30

F32 = mybir.dt.float32
BF16 = mybir.dt.bfloat16

_GRAPH_CACHE = {}
LAST_PROFILE = None


def _bcast_like(small_ap, big_ap):
    """Broadcast small_ap (size-1 dims) against big_ap's shape."""
    a, b = bass.broadcast_tensor_aps(big_ap, small_ap)
    return b


def _build_graph():
    nc = bacc.Bacc(debug=False)

    xt_ext = nc.dram_tensor("xt", [NDC, 128, S], BF16, kind="ExternalInput")
    # packed q|k|v weight columns (host-permuted evens-first for q/k)
    wqkv_ext = nc.dram_tensor("wqkv", [NDC, 128, DQ + DK + DK], BF16,
                              kind="ExternalInput")
    wo_ext = nc.dram_tensor("wo", [NH, 128, D], BF16, kind="ExternalInput")
    # rope coefficient tiles for [d, s] layout (evens-first halves):
    # cd1 = [C; C], cd2 = [-D; D] with C[i, s] = fc[s, i, 0]
    cd1_ext = nc.dram_tensor("cd1", [128, S], BF16, kind="ExternalInput")
    cd2_ext = nc.dram_tensor("cd2", [128, S], BF16, kind="ExternalInput")
    # bias column tile: col j = bias for output-column-block j (q0..q15,k0..3,v0..3)
    bqkv_ext = nc.dram_tensor("bqkv", [128, 24], F32, kind="ExternalInput")
    out_ext = nc.dram_tensor("out", [D, S], F32, kind="ExternalOutput")

    with tile.TileContext(nc) as tc:
        with (
            tc.tile_pool(name="const", bufs=1) as cpool,
            tc.tile_pool(name="persist", bufs=1) as ppool,
            tc.tile_pool(name="stat", bufs=2) as spool,
        ):
            # multiplicative maskT for aT [s2, s1]: 1 where s1 >= s2 else 0
            maskT = cpool.tile([128, 128], BF16)
            nc.gpsimd.memset(maskT[:], 1.0)
            nc.gpsimd.affine_select(
                out=maskT[:], in_=maskT[:], compare_op=mybir.AluOpType.is_ge,
                fill=0.0, base=0, pattern=[[1, 128]], channel_multiplier=-1)
            ident_b = cpool.tile([128, 128], BF16)
            make_identity(nc, ident_b)
            ones_col = cpool.tile([128, 1], BF16)   # den matmul lhsT
            nc.gpsimd.memset(ones_col[:], 1.0)
            ones_row = cpool.tile([1, 128], BF16)   # bcast matmul lhsT (K=1)
            nc.gpsimd.memset(ones_row[:], 1.0)
            cd1_sb = cpool.tile([128, S], BF16)
            nc.sync.dma_start(out=cd1_sb[:], in_=cd1_ext[:])
            cd2_sb = cpool.tile([128, S], BF16)
            nc.sync.dma_start(out=cd2_sb[:], in_=cd2_ext[:])
            bias_sb = cpool.tile([128, 24], F32)
            nc.sync.dma_start(out=bias_sb[:], in_=bqkv_ext[:])

            # persistent activations
            qT_all = ppool.tile([128, NH * S], BF16)    # [d, h*S + s]
            kT_all = ppool.tile([128, NKV * S], BF16)   # [d, g*S + s]
            v_all = ppool.tile([128, NSC * DK], BF16)   # [s2 in chunk, sc*DK + d]

            # ---------------- Phase 1: QKV + rope ----------------
            # W-stationary: for each output-column block (24 blocks of 128:
            # 4 k-heads, 4 v-heads, 16 q-heads) accumulate psum [128, S] over
            # the 32 D-chunks with x (bf16, SBUF-resident) as moving operand.
            # Output arrives already transposed ([d, s]). RoPE runs directly
            # in [d, s] layout: the host permutes q/k weight columns so even
            # rope components land on partitions 0:64 and odd on 64:128; a
            # partition-swapping SBUF->SBUF DMA provides the crossed term, and
            # host-precomputed [C;C] / [-D;D] coefficient tiles feed 3 DVE
            # tensor_tensor ops that write qT/kT directly. v is PE-transposed
            # into [s2, d] layout.
            with (
                tc.tile_pool(name="p1sb", bufs=1) as p1,
                tc.tile_pool(name="p1ps", bufs=1, space="PSUM") as ps1,
            ):
                # x tiles are DMA'd lazily inside group 0's dc loop so the
                # first group's weight DMAs aren't queued behind all of x
                xts = [
                    p1.tile([128, S], BF16, tag=f"xt{dc}", name=f"xt{dc}")
                    for dc in range(NDC)
                ]

                for grp in range(8):
                    accs = [
                        ps1.tile([128, S], F32, tag="acc", bufs=3, name=f"acc{grp}_{b}")
                        for b in range(3)
                    ]
                    for dc in range(NDC):
                        if grp == 0:
                            nc.sync.dma_start(out=xts[dc][:], in_=xt_ext[dc])
                        w_t = p1.tile([128, 384], BF16, tag="w", bufs=4,
                                      name=f"w{grp}_{dc}")
                        nc.sync.dma_start(
                            out=w_t[:],
                            in_=wqkv_ext[dc, :, grp * 384:(grp + 1) * 384])
                        for b in range(3):
                            lhs = w_t[:, b * 128:(b + 1) * 128]
                            for c0 in range(0, S, 512):
                                nc.tensor.matmul(
                                    accs[b][:, c0:c0 + 512], lhs,
                                    xts[dc][:, c0:c0 + 512],
                                    start=(dc == 0), stop=(dc == NDC - 1))
                    for b in range(3):
                        blk = grp * 3 + b
                        # evict + bias (per-partition col) -> bf16 [d, s]
                        tf = p1.tile([128, S], BF16, tag="tf", bufs=3,
                                     name=f"tf{blk}")
                        nc.vector.tensor_scalar(
                            out=tf[:], in0=accs[b][:],
                            scalar1=bias_sb[:, blk:blk + 1],
                            scalar2=None, op0=mybir.AluOpType.add)
                        if 4 <= blk < 8:
                            # v block: PE-transpose into [s2, d] layout
                            g = blk - 4
                            for sc in range(NSC):
                                tp = ps1.tile([128, 128], BF16, tag="tp", bufs=2,
                                              name=f"tp{blk}_{sc}")
                                nc.tensor.transpose(
                                    tp[:], tf[:, sc * 128:(sc + 1) * 128], ident_b)
                                nc.scalar.copy(
                                    v_all[:, sc * DK + g * 128:
                                          sc * DK + (g + 1) * 128],
                                    tp[:])
                        else:
                            # q/k block: rope in [d, s] layout
                            if blk < 4:
                                dstT, idx = kT_all, blk
                            else:
                                dstT, idx = qT_all, blk - 8
                            tfs = p1.tile([128, S], BF16, tag="tfs", bufs=3,
                                          name=f"tfs{blk}")
                            nc.gpsimd.dma_start(out=tfs[0:64, :], in_=tf[64:128, :])
                            nc.gpsimd.dma_start(out=tfs[64:128, :], in_=tf[0:64, :])
                            t1 = p1.tile([128, S], BF16, tag="rt0", bufs=3,
                                         name=f"rt0_{blk}")
                            nc.vector.tensor_tensor(
                                out=t1[:], in0=tf[:], in1=cd1_sb[:],
                                op=mybir.AluOpType.mult)
                            t2 = p1.tile([128, S], BF16, tag="rt1", bufs=3,
                                         name=f"rt1_{blk}")
                            nc.vector.tensor_tensor(
                                out=t2[:], in0=tfs[:], in1=cd2_sb[:],
                                op=mybir.AluOpType.mult)
                            nc.vector.tensor_tensor(
                                out=dstT[:, idx * S:(idx + 1) * S],
                                in0=t1[:], in1=t2[:], op=mybir.AluOpType.add)

            # ---------------- Phase 2: attention ----------------
            # oT_all lives through phases 2+3; released manually at the end.
            p23 = tc.alloc_tile_pool(name="p23sb", bufs=1)
            oT_all = p23.tile([128, NH * S], BF16, name="oT_all")  # [d, h*S+s]
            with (
                tc.tile_pool(name="p2sb", bufs=1) as p2,
                tc.tile_pool(name="p2ps", bufs=1, space="PSUM") as ps2,
            ):
                for h in range(NH):
                    g = h // 4
                    # scoresT_j = kT_j.T @ qT (causal: s1 >= j*128), exp -> aT,
                    # then av + den accumulate immediately
                    otp = ps2.tile([128, 1024], F32, tag="ot", bufs=2, name=f"otp{h}")
                    den = ps2.tile([1, 1024], F32, tag="ot", bufs=2, name=f"den{h}")
                    for j in range(NSC):
                        if j < 4:
                            chunks = [(j * 128, 512), (512, 1024)]
                        else:
                            chunks = [(j * 128, 1024)]
                        scp = ps2.tile([128, 1024], F32, tag="sc", bufs=2,
                                       name=f"scp{h}_{j}")
                        lhs = kT_all[:, g * S + j * 128: g * S + (j + 1) * 128]
                        for (c0, c1) in chunks:
                            nc.tensor.matmul(
                                scp[:, c0:c1], lhs,
                                qT_all[:, h * S + c0: h * S + c1],
                                start=True, stop=True)
                        aT = p2.tile([128, 1024], BF16, tag="aT", bufs=4,
                                     name=f"aT{h}_{j}")
                        vs = v_all[:, j * DK + g * 128: j * DK + (g + 1) * 128]
                        # per-512-chunk exp -> mask (diag chunk) -> av + den,
                        # so downstream work starts before the full row is
                        # exponentiated
                        for ci, (c0, c1) in enumerate(chunks):
                            nc.scalar.activation(
                                aT[:, c0:c1], scp[:, c0:c1],
                                mybir.ActivationFunctionType.Exp,
                                scale=QK_SCALE)
                            if ci == 0:
                                # causal mask on diagonal block
                                nc.vector.tensor_tensor(
                                    out=aT[:, j * 128:(j + 1) * 128],
                                    in0=aT[:, j * 128:(j + 1) * 128],
                                    in1=maskT[:], op=mybir.AluOpType.mult)
                            nc.tensor.matmul(
                                otp[:, c0:c1], vs, aT[:, c0:c1],
                                start=(j == 0), stop=(j == NSC - 1),
                                skip_group_check=True)
                            nc.tensor.matmul(
                                den[:, c0:c1], ones_col[:], aT[:, c0:c1],
                                start=(j == 0), stop=(j == NSC - 1),
                                skip_group_check=True)

                    # normalize: den -> bf16 row, PE-broadcast to 128
                    # partitions, full-lane reciprocal, multiply during
                    # psum->sbuf eviction
                    drow = spool.tile([1, 1024], BF16, tag="drow", name=f"dr{h}")
                    nc.scalar.copy(drow[:], den[:])
                    rb = ps2.tile([128, 1024], F32, tag="sc", bufs=2, name=f"rb{h}")
                    for c0 in range(0, S, 512):
                        nc.tensor.matmul(rb[:, c0:c0 + 512], ones_row[:],
                                         drow[:, c0:c0 + 512],
                                         start=True, stop=True)
                    rbs = p2.tile([128, 1024], F32, tag="rbs", bufs=2, name=f"rbs{h}")
                    nc.vector.reciprocal_approx_fast(out=rbs[:], in_=rb[:])
                    nc.vector.tensor_tensor(
                        out=oT_all[:, h * S:(h + 1) * S], in0=otp[:], in1=rbs[:],
                        op=mybir.AluOpType.mult)

            # ---------------- Phase 3: Wo ----------------
            # Groups of 4 Dout-chunks x both s-halves (8 psum banks); each
            # stationary Wo block serves both halves, Wo is streamed once.
            with (
                tc.tile_pool(name="p3sb", bufs=1) as p3,
                tc.tile_pool(name="p3ps", bufs=1, space="PSUM") as ps3,
            ):
                # 16 groups of 2 Dout-chunks x 2 s-halves (4 psum banks per
                # group, bufs=8 -> two groups in flight)
                for mg in range(16):
                    wps = [
                        ps3.tile([128, 512], F32, tag="wps", bufs=8,
                                 name=f"wps{mg}_{i}")
                        for i in range(4)  # [m0s0, m0s1, m1s0, m1s1]
                    ]
                    for c in range(NH):
                        wo_t = p3.tile([128, 256], BF16, tag="wo", bufs=8,
                                       name=f"wo{mg}_{c}")
                        nc.gpsimd.dma_start(
                            out=wo_t[:],
                            in_=wo_ext[c, :, mg * 256:(mg + 1) * 256])
                        for m in range(2):
                            lhs = wo_t[:, m * 128:(m + 1) * 128]
                            for sh in range(2):
                                rhs = oT_all[:, c * S + sh * 512:
                                             c * S + sh * 512 + 512]
                                nc.tensor.matmul(
                                    wps[m * 2 + sh][:], lhs, rhs,
                                    start=(c == 0), stop=(c == NH - 1))
                    for m in range(2):
                        for sh in range(2):
                            ot_sb = p3.tile([128, 512], F32, tag="ot_sb", bufs=4,
                                            name=f"osb{mg}_{m}_{sh}")
                            nc.vector.tensor_copy(ot_sb[:], wps[m * 2 + sh][:])
                            mm = mg * 2 + m
                            nc.sync.dma_start(
                                out=out_ext[mm * 128:(mm + 1) * 128,
                                            sh * 512:(sh + 1) * 512],
                                in_=ot_sb[:])
            p23.release()

    nc.compile()
    return nc


def _evens_first_perm(nheads):
    idx = []
    for h in range(nheads):
        base = h * HD
        idx.extend(range(base, base + HD, 2))
        idx.extend(range(base + 1, base + HD, 2))
    return np.array(idx, dtype=np.int64)


def kernel(x, freqs_cis, Wq, bq, Wk, bk, Wv, bv, Wo, bo, startpos):
    global LAST_PROFILE
    x = np.asarray(x, dtype=np.float32)
    freqs_cis = np.asarray(freqs_cis, dtype=np.float32)
    Wq = np.asarray(Wq, dtype=np.float32)
    Wk = np.asarray(Wk, dtype=np.float32)
    Wv = np.asarray(Wv, dtype=np.float32)
    Wo = np.asarray(Wo, dtype=np.float32)
    bq = np.asarray(bq, dtype=np.float32)
    bk = np.asarray(bk, dtype=np.float32)
    bv = np.asarray(bv, dtype=np.float32)
    bo = np.asarray(bo, dtype=np.float32)
    assert int(startpos) == 0

    bf = lambda a: np.ascontiguousarray(a.astype(ml_dtypes.bfloat16))
    f32c = lambda a: np.ascontiguousarray(a.astype(np.float32))

    # rope coefficients in [d, s] layout: C64[i, s] = fc[s, i, 0]
    C64 = freqs_cis[:, :, 0].T          # [64, S]
    D64 = freqs_cis[:, :, 1].T
    cd1 = bf(np.vstack([C64, C64]))     # coeff on tf   (rows 0:64 = E, 64: = O)
    cd2 = bf(np.vstack([-D64, D64]))    # coeff on swapped tf

    in_maps = []
    for core in range(8):
        b, g = core // 2, core % 2
        qsel = g * DQ + _evens_first_perm(NH)
        ksel = g * DK + _evens_first_perm(NKV)
        vsel = np.arange(g * DK, (g + 1) * DK)
        if core < 2:  # weight shards depend only on g; reuse for later cores
            wqkv_h = bf(np.concatenate(
                [Wk[:, ksel], Wv[:, vsel], Wq[:, qsel]], 1
            ).reshape(NDC, 128, DQ + DK + DK))
            wo_h = bf(Wo[g * DQ:(g + 1) * DQ, :].reshape(NH, 128, D))
            bqkv = np.concatenate([bk[ksel], bv[vsel], bq[qsel]])
            bqkv = f32c(bqkv.reshape(24, 128).T)  # [128, 24]: col j = block j bias
        else:
            prev = in_maps[core - 2]
            wqkv_h, wo_h, bqkv = prev["wqkv"], prev["wo"], prev["bqkv"]
        xt_h = bf(x[b].T.reshape(NDC, 128, S))
        in_maps.append({
            "xt": xt_h, "wqkv": wqkv_h, "wo": wo_h,
            "cd1": cd1, "cd2": cd2, "bqkv": bqkv,
        })

    if "nc" not in _GRAPH_CACHE:
        _GRAPH_CACHE["nc"] = _build_graph()
    nc = _GRAPH_CACHE["nc"]

    res = run_bass_kernel_spmd(
        nc, in_maps, core_ids=list(range(8)),
        trace=bool(os.environ.get("BASS_TRACE")))
    LAST_PROFILE = res

    out = np.empty((B, S, D), dtype=np.float32)
    for b in range(B):
        t = res.results[2 * b]["out"] + res.results[2 * b + 1]["out"]
        out[b] = t.T + bo[None, :]
    return out
